# revision 22
# baseline (speedup 1.0000x reference)
"""
DeepAttMISL segment-reduce kernel for Trainium2 (Bass/Tile), 8 NeuronCores.

Math (see reference):
  h        = relu(x @ W1.T + b1)                    x:[N,1024] -> h:[N,256]
  seg      = segment_sum(h, cluster_id, 8)          -> [8,256]
  h_clust  = seg / max(counts,1)
  h_path   = relu(h_clust @ Wf.T + bf)
  A        = softmax((tanh(h_path@Wa.T+ba) * sigmoid(h_path@Wb.T+bb)) @ Wc.T)
  H        = A @ h_path                             -> [1,256]

Sharding: BY CLUSTER, not by rows.  Core k receives ALL rows of cluster k
(host sorts rows by cluster_id), zero-padded to a fixed NPAD rows.  Each
core therefore owns its cluster's full segment sum locally and NO cross-core
collective is needed (the ncfw AllReduce costs 25-35us per op in this
runtime, plus a ~56us wake, and dominated the previous version's critical
path).  Each core runs the tiny gated-attention head for its own cluster and
outputs (logit_k, h_path_k); the host does the final 8-way softmax +
weighted sum as the gather/unshard step.

Main matmul is computed TRANSPOSED (W1 stationary, x moving, h.T in PSUM
[hid_half, rows]) so the segment sum falls out of ACT's accum_out: one
activation op per PSUM tile does bias + relu + sum-over-rows.  No segment
matmuls, no one-hot matrix.  Zero-pad rows contribute exactly relu(b1)
each; the host bakes -n_pad*relu(b1)/count into a per-core correction.

x is streamed as NBLK contiguous 1MiB DMAs (8KiB per partition per block)
on the sync ring - near line rate.  bf16 everywhere in the big matmul
(fp8 fails the 2e-2 gate: W1's quantization error is shared across
instances so it does not average out); fp32 head.  sigmoid(y) =
0.5*(1+tanh(y/2)) with the 0.5 folded into Wc so one ACT table set
(relu/tanh/exp) serves the whole kernel.
"""

import sys

if "/opt/trn_rl_repo" not in sys.path:
    sys.path.insert(0, "/opt/trn_rl_repo")

import numpy as np
import ml_dtypes

import concourse.bass as bass
import concourse.tile as tile
from concourse import bacc, mybir
from concourse import bass_utils

ALU = mybir.AluOpType

N_CORES = 8
N_TOTAL = 65536
DIN = 1024
DHID = 256
K_CL = 8
KC = DIN // 128                        # 8 contraction chunks of 128
BLK = 448                              # rows per block (<=512 fp32 PSUM bank;
                                       # 448 cuts zero-padding to ~2%)
WARMUP_MMS = 13                        # PE bridge: engine free ~8.2us, block0
                                       # ~13.2us; also >=3.4us busy for HAM

# wblob: head weights, float32r (PE-only consumers), per-partition fp32 elems
OFF_WFT = 0                            # Wf.T tiled   [2,256] -> 512
OFF_WAT = 512                          # Wa.T tiled   [2,256] -> 512
OFF_WBT = 1024                         # (Wb/2).T     [2,256] -> 512
OFF_WCR = 1536                         # (Wc/2) bcast [2,128] -> 256
OFF_BFR = 1792                         # bf as a row (bcast to all p) -> 256
OFF_BAR = 2048                         # ba row                       -> 256
OFF_BBR = 2304                         # bb/2 row                     -> 256
OFF_ONE = 2560                         # constant 1.0 x2 (rhs of bias MMs)
NWBLOB = 2562
# sblob: fp32 scalars for ACT bias / DVE (b1 tiled, padding fix, 1/count)
OFF_B1C = 0                            # b1 tiled [2]
OFF_CORR = 2                           # -invc*n_pad*relu(b1), dup pairs [4]
OFF_INVC = 6                           # 1/max(count,1) scalar [1]
NSBLOB = 7

BF16 = mybir.dt.bfloat16
F32 = mybir.dt.float32
AF = mybir.ActivationFunctionType

_CACHE = {}


def _build_nc(nblk):
    npad = nblk * BLK
    nc = bacc.Bacc("TRN2", target_bir_lowering=False, debug=False,
                   num_devices=N_CORES)

    xb = nc.dram_tensor("xb", [128, nblk * KC * BLK], BF16,
                        kind="ExternalInput")
    w1t = nc.dram_tensor("w1t", [128, KC * DHID], BF16, kind="ExternalInput")
    wblob = nc.dram_tensor("wblob", [128, NWBLOB], mybir.dt.float32r,
                           kind="ExternalInput")
    sblob = nc.dram_tensor("sblob", [128, NSBLOB], F32, kind="ExternalInput")
    out = nc.dram_tensor("out", [128, 6], F32, kind="ExternalOutput")

    with tile.TileContext(nc) as tc:
        with tc.tile_pool(name="consts", bufs=1) as consts, \
             tc.tile_pool(name="xblk", bufs=1) as xblk, \
             tc.tile_pool(name="hps", bufs=4, space="PSUM") as hps, \
             tc.tile_pool(name="headps", bufs=2, space="PSUM") as headps, \
             tc.tile_pool(name="small", bufs=1) as small:

            # ---- PE warm-up bridge: keep HAM busy (and un-throttled by the
            # time real data arrives) from t~0 until block 0 lands (~13us).
            wz = consts.tile([128, BLK], BF16)
            nc.vector.memset(wz[:], 0.0)
            wps = hps.tile([128, BLK], F32, tag="main")
            for _ in range(WARMUP_MMS):
                nc.tensor.matmul(wps[:], wz[:, 0:128], wz[:],
                                 start=True, stop=True, skip_group_check=True)

            # ---- DMAs: ALL on the sync ring, in consumption order.  The
            # tiny sblob goes first purely to eat the ~1us cold-first-
            # descriptor cost on all 16 SDMA engines; then W1 (gates the
            # first MM), then the x blocks as back-to-back ~0.9MiB
            # transfers (7KiB/partition lines, full line rate), and the
            # head-weight blob LAST (needed only ~60us in; anywhere earlier
            # it steals SDMA bandwidth from block 0 and delays the whole
            # pipeline).
            F32R = mybir.dt.float32r
            sblob_sb = consts.tile([128, NSBLOB], F32)
            nc.sync.dma_start(sblob_sb[:], sblob.ap())
            w1t_sb = consts.tile([128, KC, DHID], BF16)
            nc.sync.dma_start(w1t_sb[:], w1t.ap().rearrange(
                "p (k f) -> p k f", k=KC))

            xts = xblk.tile([128, nblk, KC, BLK], BF16)
            xv = xb.ap().rearrange("p (b k r) -> p b k r", b=nblk, k=KC)
            for b in range(nblk):
                nc.sync.dma_start(xts[:, b], xv[:, b])
            wblob_sb = consts.tile([128, NWBLOB], F32R)
            nc.sync.dma_start(wblob_sb[:], wblob.ap())

            # ---- main loop: per block, per hid-half: 8 accumulated MMs
            # (W1 chunk stationary, x moving, N=512), then one ACT op doing
            # bias + relu + accum_out (the segment sum over this block).
            segparts = small.tile([128, 2, nblk], F32)
            hsc = [small.tile([128, BLK], BF16, name=f"hsc{i}")
                   for i in range(3)]
            for b in range(nblk):
                for j in range(2):
                    ps = hps.tile([128, BLK], F32, tag="main")
                    for c in range(KC):
                        nc.tensor.matmul(
                            ps[:],
                            w1t_sb[:, c, j * 128:(j + 1) * 128],
                            xts[:, b, c, :],
                            start=(c == 0), stop=(c == KC - 1),
                            skip_group_check=True)
                    nc.scalar.activation(
                        hsc[(2 * b + j) % 3][:], ps[:], AF.Relu,
                        bias=sblob_sb[:, OFF_B1C + j:OFF_B1C + j + 1],
                        accum_out=segparts[:, j, b:b + 1])

            # ---- local segment sum -> cluster mean (with padding fix).
            # Columns come in duplicated pairs (cols 2j and 2j+1 equal):
            # the fp32r matmul ISA requires an even moving free dim, so the
            # whole head works on [128, 4] with N=2 GEMVs.
            seg4 = small.tile([128, 4], F32)
            for j in range(2):
                nc.vector.reduce_sum(seg4[:, 2 * j:2 * j + 1],
                                     segparts[:, j, :],
                                     axis=mybir.AxisListType.X)
                nc.vector.tensor_copy(seg4[:, 2 * j + 1:2 * j + 2],
                                      seg4[:, 2 * j:2 * j + 1])
            hc = small.tile([128, 4], F32R)
            nc.vector.tensor_scalar_mul(hc[:], seg4[:],
                                        sblob_sb[:, OFF_INVC:OFF_INVC + 1])
            nc.vector.tensor_add(hc[:], hc[:],
                                 sblob_sb[:, OFF_CORR:OFF_CORR + 4])

            # ---- gated-attention head for this core's cluster ----
            # fp32r matmuls (1-pass fp22, single LDW pass vs fp32's two of
            # each); bias folded into the accumulation group as a K=1
            # matmul against a constant-one rhs so one ACT handles both
            # hid-halves of a layer.
            def head_layer(w_off, b_off, rhs, func, name):
                o = small.tile([128, 4], F32R, name=name)
                ps = headps.tile([128, 4], F32, tag="head",
                                 padded_shape=[128, BLK])
                n = 0
                for j in range(2):
                    for i in range(2):
                        nc.tensor.matmul(
                            ps[:, 2 * j:2 * j + 2],
                            wblob_sb[:, w_off + i * 256 + j * 128:
                                     w_off + i * 256 + (j + 1) * 128],
                            rhs[:, 2 * i:2 * i + 2],
                            start=(n == 0), stop=False,
                            skip_group_check=True)
                        n += 1
                    nc.tensor.matmul(
                        ps[:, 2 * j:2 * j + 2],
                        wblob_sb[0:1, b_off + j * 128:
                                 b_off + (j + 1) * 128],
                        wblob_sb[0:1, OFF_ONE:OFF_ONE + 2],
                        start=False, stop=(j == 1), skip_group_check=True)
                nc.scalar.activation(o[:], ps[:], func)
                return o

            hpT = head_layer(OFF_WFT, OFF_BFR, hc, AF.Relu, "hpT")
            aT = head_layer(OFF_WAT, OFF_BAR, hpT, AF.Tanh, "aT")
            tT = head_layer(OFF_WBT, OFF_BBR, hpT, AF.Tanh, "tT")
            # a*g = 0.5*a*(1+tanh(y/2)); the 0.5 lives in Wc/2
            ag = small.tile([128, 4], F32R)
            nc.vector.tensor_mul(ag[:], aT[:], tT[:])
            nc.vector.tensor_add(ag[:], ag[:], aT[:])

            # logit (replicated across partitions via broadcast Wc/2)
            lps = headps.tile([128, 2], F32, tag="head",
                              padded_shape=[128, BLK])
            for j in range(2):
                nc.tensor.matmul(
                    lps[:],
                    wblob_sb[:, OFF_WCR + j * 128:
                             OFF_WCR + (j + 1) * 128],
                    ag[:, 2 * j:2 * j + 2],
                    start=(j == 0), stop=(j == 1))

            # stream h_path out as soon as it's ready (overlaps the gate
            # matmuls); the logit follows in a second small DMA
            nc.sync.dma_start(out.ap()[:, 0:4].bitcast(F32R), hpT[:])
            lsb = small.tile([128, 2], F32)
            nc.vector.tensor_copy(lsb[:], lps[:])
            nc.sync.dma_start(out.ap()[:, 4:6], lsb[:])

    nc.compile()
    return nc


def _shard_plan(cluster_id):
    cid = np.asarray(cluster_id).astype(np.int64).reshape(N_TOTAL)
    counts = np.bincount(cid, minlength=K_CL).astype(np.int64)
    nblk = max(1, -(-int(counts.max()) // BLK))        # ceil(max/BLK)
    return cid, counts, nblk


def _prep_inputs(x_path, cluster_id, W1, b1, Wf, bf, Wa, ba, Wb, bb, Wc, bc):
    """Host-side sharding / marshalling. Returns (in_maps, nblk)."""
    cid, counts, nblk = _shard_plan(cluster_id)
    npad = nblk * BLK
    x = np.asarray(x_path, dtype=np.float32).reshape(N_TOTAL, DIN)
    xb16 = x.astype(ml_dtypes.bfloat16)

    W1 = np.asarray(W1, np.float32); b1 = np.asarray(b1, np.float32)
    Wf = np.asarray(Wf, np.float32); bf = np.asarray(bf, np.float32)
    Wa = np.asarray(Wa, np.float32); ba = np.asarray(ba, np.float32)
    Wb = np.asarray(Wb, np.float32); bb = np.asarray(bb, np.float32)
    Wc = np.asarray(Wc, np.float32)

    def tiled_T(M):  # [256,256] -> [128, 512]; [p, j*256+f] = M.T[j*128+p, f]
        return np.ascontiguousarray(
            M.T.reshape(2, 128, DHID).transpose(1, 0, 2)).reshape(128, 512)

    def tiled_v(v):  # [256] -> [128, 2]; [p, j] = v[j*128+p]
        return np.ascontiguousarray(v.reshape(2, 128).T)

    wblob = np.zeros((128, NWBLOB), np.float32)
    wblob[:, OFF_WFT:OFF_WFT + 512] = tiled_T(Wf)
    wblob[:, OFF_WAT:OFF_WAT + 512] = tiled_T(Wa)
    wblob[:, OFF_WBT:OFF_WBT + 512] = tiled_T(Wb * 0.5)
    wcr = np.broadcast_to((Wc.ravel() * 0.5).reshape(2, 128, 1),
                          (2, 128, 128)).transpose(1, 0, 2)
    wblob[:, OFF_WCR:OFF_WCR + 256] = wcr.reshape(128, 256)
    wblob[:, OFF_BFR:OFF_BFR + 256] = bf[None, :]
    wblob[:, OFF_BAR:OFF_BAR + 256] = ba[None, :]
    wblob[:, OFF_BBR:OFF_BBR + 256] = (bb * 0.5)[None, :]
    wblob[:, OFF_ONE:OFF_ONE + 2] = 1.0
    sblob_base = np.zeros((128, NSBLOB), np.float32)
    sblob_base[:, OFF_B1C:OFF_B1C + 2] = tiled_v(b1)

    # W1.T tiled: [p, c*256+m] = W1[m, c*128+p]
    w1tt = np.ascontiguousarray(
        W1.T.reshape(KC, 128, DHID).transpose(1, 0, 2)
    ).reshape(128, KC * DHID).astype(ml_dtypes.bfloat16)

    relu_b1 = np.maximum(b1, 0.0).astype(np.float32)

    in_maps = []
    for k in range(N_CORES):
        rows = np.nonzero(cid == k)[0]
        nk = len(rows)
        shard = np.zeros((npad, DIN), dtype=ml_dtypes.bfloat16)
        shard[:nk] = xb16[rows]
        # [npad, 1024] -> [p, b, c, r] -> flat [128, nblk*8*512]
        xcore = np.ascontiguousarray(
            shard.reshape(nblk, BLK, KC, 128).transpose(3, 0, 2, 1)
        ).reshape(128, nblk * KC * BLK)

        invc = np.float32(1.0 / max(float(counts[k]), 1.0))
        n_pad = float(npad - nk)
        sblob_k = sblob_base.copy()
        corr = tiled_v((-invc * n_pad) * relu_b1)     # [128, 2]
        sblob_k[:, OFF_CORR:OFF_CORR + 4] = corr[:, [0, 0, 1, 1]]
        sblob_k[:, OFF_INVC] = invc
        in_maps.append({"xb": xcore, "w1t": w1tt, "wblob": wblob,
                        "sblob": sblob_k})
    return in_maps, nblk


def kernel(**inputs):
    _, _, nblk = _shard_plan(inputs["cluster_id"])
    key = ("nc", nblk)
    if key not in _CACHE:
        _CACHE[key] = _build_nc(nblk)
        _CACHE["nc"] = _CACHE[key]       # convenience handle for test.py
    nc = _CACHE[key]
    in_maps, _ = _prep_inputs(**inputs)
    res = bass_utils.run_bass_kernel_spmd(
        nc, in_maps, core_ids=list(range(N_CORES)))
    return _combine([res.results[k]["out"] for k in range(N_CORES)])


def _combine(outs):
    """Host-side gather: softmax over per-cluster logits + weighted sum."""
    logits = np.array([float(np.asarray(o)[0, 4]) for o in outs],
                      dtype=np.float64)
    h_path = np.stack([np.asarray(o)[:, [0, 2]].T.reshape(DHID)
                       for o in outs])
    w = np.exp(logits - logits.max())
    w /= w.sum()
    H = (w[:, None] * h_path.astype(np.float64)).sum(axis=0)
    return np.ascontiguousarray(H.reshape(1, DHID)).astype(np.float32)


# revision 23
# speedup vs baseline: 1.1697x; 1.1697x over previous
"""
DeepAttMISL segment-reduce kernel for Trainium2 (Bass/Tile), 8 NeuronCores.

Math (see reference):
  h        = relu(x @ W1.T + b1)                    x:[N,1024] -> h:[N,256]
  seg      = segment_sum(h, cluster_id, 8)          -> [8,256]
  h_clust  = seg / max(counts,1)
  h_path   = relu(h_clust @ Wf.T + bf)
  A        = softmax((tanh(h_path@Wa.T+ba) * sigmoid(h_path@Wb.T+bb)) @ Wc.T)
  H        = A @ h_path                             -> [1,256]

Sharding: BY CLUSTER, not by rows.  Core k receives ALL rows of cluster k
(host sorts rows by cluster_id), zero-padded to a fixed NPAD rows.  Each
core therefore owns its cluster's full segment sum locally and NO cross-core
collective is needed (the ncfw AllReduce costs 25-35us per op in this
runtime, plus a ~56us wake, and dominated the previous version's critical
path).  Each core runs the tiny gated-attention head for its own cluster and
outputs (logit_k, h_path_k); the host does the final 8-way softmax +
weighted sum as the gather/unshard step.

Main matmul is computed TRANSPOSED (W1 stationary, x moving, h.T in PSUM
[hid_half, rows]) so the segment sum falls out of ACT's accum_out: one
activation op per PSUM tile does bias + relu + sum-over-rows.  No segment
matmuls, no one-hot matrix.  Zero-pad rows contribute exactly relu(b1)
each; the host bakes -n_pad*relu(b1)/count into a per-core correction.

x is streamed as NBLK contiguous 1MiB DMAs (8KiB per partition per block)
on the sync ring - near line rate.  bf16 everywhere in the big matmul
(fp8 fails the 2e-2 gate: W1's quantization error is shared across
instances so it does not average out); fp32 head.  sigmoid(y) =
0.5*(1+tanh(y/2)) with the 0.5 folded into Wc so one ACT table set
(relu/tanh/exp) serves the whole kernel.
"""

import sys

if "/opt/trn_rl_repo" not in sys.path:
    sys.path.insert(0, "/opt/trn_rl_repo")

import numpy as np
import ml_dtypes

import concourse.bass as bass
import concourse.tile as tile
from concourse import bacc, mybir
from concourse import bass_utils

ALU = mybir.AluOpType

N_CORES = 8
N_TOTAL = 65536
DIN = 1024
DHID = 256
K_CL = 8
KC = DIN // 128                        # 8 contraction chunks of 128
BLK = 448                              # bulk rows per block (<=512 fp32 PSUM
                                       # bank; 448 cuts zero-padding to ~2%)
BLK0 = 224                             # first/last block halved: block 0
                                       # completes its DMA sooner (earlier PE
                                       # start), same total padding
WARMUP_MMS = 11                        # PE bridge: engine free ~8.2us, block0
                                       # ~12.5us; also >=3.4us busy for HAM

# wblob: head weights, float32r (PE-only consumers), per-partition fp32 elems
OFF_WFT = 0                            # Wf.T tiled   [2,256] -> 512
OFF_WAT = 512                          # Wa.T tiled   [2,256] -> 512
OFF_WBT = 1024                         # (Wb/2).T     [2,256] -> 512
OFF_WCR = 1536                         # (Wc/2) bcast [2,128] -> 256
OFF_BFR = 1792                         # bf as a row (bcast to all p) -> 256
OFF_BAR = 2048                         # ba row                       -> 256
OFF_BBR = 2304                         # bb/2 row                     -> 256
OFF_ONE = 2560                         # constant 1.0 x2 (rhs of bias MMs)
NWBLOB = 2562
# sblob: fp32 scalars for ACT bias / DVE (b1 tiled, padding fix, 1/count)
OFF_B1C = 0                            # b1 tiled [2]
OFF_CORR = 2                           # -invc*n_pad*relu(b1), dup pairs [4]
OFF_INVC = 6                           # 1/max(count,1) scalar [1]
NSBLOB = 7

BF16 = mybir.dt.bfloat16
F32 = mybir.dt.float32
AF = mybir.ActivationFunctionType

_CACHE = {}


def _build_nc(sizes):
    nblk = len(sizes)
    offs = [0]
    for s in sizes:
        offs.append(offs[-1] + s)
    tot = offs[-1]
    nc = bacc.Bacc("TRN2", target_bir_lowering=False, debug=False,
                   num_devices=N_CORES)

    xb = nc.dram_tensor("xb", [128, KC * tot], BF16,
                        kind="ExternalInput")
    w1t = nc.dram_tensor("w1t", [128, KC * DHID], BF16, kind="ExternalInput")
    wblob = nc.dram_tensor("wblob", [128, NWBLOB], mybir.dt.float32r,
                           kind="ExternalInput")
    sblob = nc.dram_tensor("sblob", [128, NSBLOB], F32, kind="ExternalInput")
    out = nc.dram_tensor("out", [128, 6], F32, kind="ExternalOutput")

    with tile.TileContext(nc) as tc:
        with tc.tile_pool(name="consts", bufs=1) as consts, \
             tc.tile_pool(name="xblk", bufs=1) as xblk, \
             tc.tile_pool(name="hps", bufs=4, space="PSUM") as hps, \
             tc.tile_pool(name="headps", bufs=2, space="PSUM") as headps, \
             tc.tile_pool(name="small", bufs=1) as small:

            # ---- PE warm-up bridge: keep HAM busy (and un-throttled by the
            # time real data arrives) from t~0 until block 0 lands (~13us).
            wz = consts.tile([128, BLK], BF16)
            nc.vector.memset(wz[:], 0.0)
            wps = hps.tile([128, BLK], F32, tag="main")
            for _ in range(WARMUP_MMS):
                nc.tensor.matmul(wps[:], wz[:, 0:128], wz[:],
                                 start=True, stop=True, skip_group_check=True)

            # ---- DMAs: ALL on the sync ring, in consumption order.  The
            # tiny sblob goes first purely to eat the ~1us cold-first-
            # descriptor cost on all 16 SDMA engines; then W1 (gates the
            # first MM), then the x blocks as back-to-back ~0.9MiB
            # transfers (7KiB/partition lines, full line rate), and the
            # head-weight blob LAST (needed only ~60us in; anywhere earlier
            # it steals SDMA bandwidth from block 0 and delays the whole
            # pipeline).
            F32R = mybir.dt.float32r
            w1t_sb = consts.tile([128, KC, DHID], BF16)
            w1v = w1t.ap().rearrange("p (k f) -> p k f", k=KC)
            # W1 in two pieces: the first eats the SDMA engines' cold
            # first-descriptor latency (~0.6-1us) while doing useful work
            nc.sync.dma_start(w1t_sb[:, 0:2], w1v[:, 0:2])
            nc.sync.dma_start(w1t_sb[:, 2:KC], w1v[:, 2:KC])

            xts = xblk.tile([128, KC * tot], BF16)
            sblob_sb = consts.tile([128, NSBLOB], F32)
            for b in range(nblk):
                nc.sync.dma_start(
                    xts[:, KC * offs[b]:KC * offs[b + 1]],
                    xb.ap()[:, KC * offs[b]:KC * offs[b + 1]])
                if b == 0:
                    nc.sync.dma_start(sblob_sb[:], sblob.ap())
            wblob_sb = consts.tile([128, NWBLOB], F32R)
            nc.sync.dma_start(wblob_sb[:], wblob.ap())

            # ---- main loop: per block, per hid-half: 8 accumulated MMs
            # (W1 chunk stationary, x moving, N=512), then one ACT op doing
            # bias + relu + accum_out (the segment sum over this block).
            segparts = small.tile([128, 2, nblk], F32)
            hsc = [small.tile([128, BLK], BF16, name=f"hsc{i}")
                   for i in range(3)]
            for b in range(nblk):
                sz, off = sizes[b], offs[b]
                for j in range(2):
                    ps = hps.tile([128, sz], F32, tag="main",
                                  padded_shape=[128, 512])
                    for c in range(KC):
                        nc.tensor.matmul(
                            ps[:],
                            w1t_sb[:, c, j * 128:(j + 1) * 128],
                            xts[:, KC * off + c * sz:KC * off + (c + 1) * sz],
                            start=(c == 0), stop=(c == KC - 1),
                            skip_group_check=True)
                    nc.scalar.activation(
                        hsc[(2 * b + j) % 3][:, 0:sz], ps[:], AF.Relu,
                        bias=sblob_sb[:, OFF_B1C + j:OFF_B1C + j + 1],
                        accum_out=segparts[:, j, b:b + 1])

            # ---- local segment sum -> cluster mean (with padding fix).
            # Columns come in duplicated pairs (cols 2j and 2j+1 equal):
            # the fp32r matmul ISA requires an even moving free dim, so the
            # whole head works on [128, 4] with N=2 GEMVs.
            seg4 = small.tile([128, 4], F32)
            for j in range(2):
                nc.vector.reduce_sum(seg4[:, 2 * j:2 * j + 1],
                                     segparts[:, j, :],
                                     axis=mybir.AxisListType.X)
                nc.vector.tensor_copy(seg4[:, 2 * j + 1:2 * j + 2],
                                      seg4[:, 2 * j:2 * j + 1])
            hc = small.tile([128, 4], F32R)
            nc.vector.tensor_scalar_mul(hc[:], seg4[:],
                                        sblob_sb[:, OFF_INVC:OFF_INVC + 1])
            nc.vector.tensor_add(hc[:], hc[:],
                                 sblob_sb[:, OFF_CORR:OFF_CORR + 4])

            # ---- gated-attention head for this core's cluster ----
            # fp32r matmuls (1-pass fp22, single LDW pass vs fp32's two of
            # each); bias folded into the accumulation group as a K=1
            # matmul against a constant-one rhs so one ACT handles both
            # hid-halves of a layer.
            def head_layer(w_off, b_off, rhs, func, name):
                o = small.tile([128, 4], F32R, name=name)
                ps = headps.tile([128, 4], F32, tag="head",
                                 padded_shape=[128, BLK])
                n = 0
                for j in range(2):
                    for i in range(2):
                        nc.tensor.matmul(
                            ps[:, 2 * j:2 * j + 2],
                            wblob_sb[:, w_off + i * 256 + j * 128:
                                     w_off + i * 256 + (j + 1) * 128],
                            rhs[:, 2 * i:2 * i + 2],
                            start=(n == 0), stop=False,
                            skip_group_check=True)
                        n += 1
                    nc.tensor.matmul(
                        ps[:, 2 * j:2 * j + 2],
                        wblob_sb[0:1, b_off + j * 128:
                                 b_off + (j + 1) * 128],
                        wblob_sb[0:1, OFF_ONE:OFF_ONE + 2],
                        start=False, stop=(j == 1), skip_group_check=True)
                nc.scalar.activation(o[:], ps[:], func)
                return o

            hpT = head_layer(OFF_WFT, OFF_BFR, hc, AF.Relu, "hpT")
            aT = head_layer(OFF_WAT, OFF_BAR, hpT, AF.Tanh, "aT")
            tT = head_layer(OFF_WBT, OFF_BBR, hpT, AF.Tanh, "tT")
            # a*g = 0.5*a*(1+tanh(y/2)); the 0.5 lives in Wc/2
            ag = small.tile([128, 4], F32R)
            nc.vector.tensor_mul(ag[:], aT[:], tT[:])
            nc.vector.tensor_add(ag[:], ag[:], aT[:])

            # logit (replicated across partitions via broadcast Wc/2)
            lps = headps.tile([128, 2], F32, tag="head",
                              padded_shape=[128, BLK])
            for j in range(2):
                nc.tensor.matmul(
                    lps[:],
                    wblob_sb[:, OFF_WCR + j * 128:
                             OFF_WCR + (j + 1) * 128],
                    ag[:, 2 * j:2 * j + 2],
                    start=(j == 0), stop=(j == 1))

            # stream h_path out as soon as it's ready (overlaps the gate
            # matmuls); the logit follows in a second small DMA
            nc.sync.dma_start(out.ap()[:, 0:4].bitcast(F32R), hpT[:])
            lsb = small.tile([128, 2], F32)
            nc.vector.tensor_copy(lsb[:], lps[:])
            nc.sync.dma_start(out.ap()[:, 4:6], lsb[:])

    nc.compile()
    return nc


def _shard_plan(cluster_id):
    cid = np.asarray(cluster_id).astype(np.int64).reshape(N_TOTAL)
    counts = np.bincount(cid, minlength=K_CL).astype(np.int64)
    mx = int(counts.max())
    k = max(0, -(-(mx - 2 * BLK0) // BLK))             # ceil((mx-448)/448)
    sizes = (BLK0,) + (BLK,) * k + (BLK0,)
    return cid, counts, sizes


def _prep_inputs(x_path, cluster_id, W1, b1, Wf, bf, Wa, ba, Wb, bb, Wc, bc):
    """Host-side sharding / marshalling. Returns (in_maps, sizes)."""
    cid, counts, sizes = _shard_plan(cluster_id)
    npad = sum(sizes)
    x = np.asarray(x_path, dtype=np.float32).reshape(N_TOTAL, DIN)
    xb16 = x.astype(ml_dtypes.bfloat16)

    W1 = np.asarray(W1, np.float32); b1 = np.asarray(b1, np.float32)
    Wf = np.asarray(Wf, np.float32); bf = np.asarray(bf, np.float32)
    Wa = np.asarray(Wa, np.float32); ba = np.asarray(ba, np.float32)
    Wb = np.asarray(Wb, np.float32); bb = np.asarray(bb, np.float32)
    Wc = np.asarray(Wc, np.float32)

    def tiled_T(M):  # [256,256] -> [128, 512]; [p, j*256+f] = M.T[j*128+p, f]
        return np.ascontiguousarray(
            M.T.reshape(2, 128, DHID).transpose(1, 0, 2)).reshape(128, 512)

    def tiled_v(v):  # [256] -> [128, 2]; [p, j] = v[j*128+p]
        return np.ascontiguousarray(v.reshape(2, 128).T)

    wblob = np.zeros((128, NWBLOB), np.float32)
    wblob[:, OFF_WFT:OFF_WFT + 512] = tiled_T(Wf)
    wblob[:, OFF_WAT:OFF_WAT + 512] = tiled_T(Wa)
    wblob[:, OFF_WBT:OFF_WBT + 512] = tiled_T(Wb * 0.5)
    wcr = np.broadcast_to((Wc.ravel() * 0.5).reshape(2, 128, 1),
                          (2, 128, 128)).transpose(1, 0, 2)
    wblob[:, OFF_WCR:OFF_WCR + 256] = wcr.reshape(128, 256)
    wblob[:, OFF_BFR:OFF_BFR + 256] = bf[None, :]
    wblob[:, OFF_BAR:OFF_BAR + 256] = ba[None, :]
    wblob[:, OFF_BBR:OFF_BBR + 256] = (bb * 0.5)[None, :]
    wblob[:, OFF_ONE:OFF_ONE + 2] = 1.0
    sblob_base = np.zeros((128, NSBLOB), np.float32)
    sblob_base[:, OFF_B1C:OFF_B1C + 2] = tiled_v(b1)

    # W1.T tiled: [p, c*256+m] = W1[m, c*128+p]
    w1tt = np.ascontiguousarray(
        W1.T.reshape(KC, 128, DHID).transpose(1, 0, 2)
    ).reshape(128, KC * DHID).astype(ml_dtypes.bfloat16)

    relu_b1 = np.maximum(b1, 0.0).astype(np.float32)

    in_maps = []
    for k in range(N_CORES):
        rows = np.nonzero(cid == k)[0]
        nk = len(rows)
        shard = np.zeros((npad, DIN), dtype=ml_dtypes.bfloat16)
        shard[:nk] = xb16[rows]
        # per block: [sz, 1024] -> [p, c, r]; concat along cols
        parts, off = [], 0
        for sz in sizes:
            parts.append(shard[off:off + sz].reshape(sz, KC, 128)
                         .transpose(2, 1, 0).reshape(128, KC * sz))
            off += sz
        xcore = np.ascontiguousarray(np.concatenate(parts, axis=1))

        invc = np.float32(1.0 / max(float(counts[k]), 1.0))
        n_pad = float(npad - nk)
        sblob_k = sblob_base.copy()
        corr = tiled_v((-invc * n_pad) * relu_b1)     # [128, 2]
        sblob_k[:, OFF_CORR:OFF_CORR + 4] = corr[:, [0, 0, 1, 1]]
        sblob_k[:, OFF_INVC] = invc
        in_maps.append({"xb": xcore, "w1t": w1tt, "wblob": wblob,
                        "sblob": sblob_k})
    return in_maps, sizes


def kernel(**inputs):
    _, _, sizes = _shard_plan(inputs["cluster_id"])
    key = ("nc", sizes)
    if key not in _CACHE:
        _CACHE[key] = _build_nc(sizes)
        _CACHE["nc"] = _CACHE[key]       # convenience handle for test.py
    nc = _CACHE[key]
    in_maps, _ = _prep_inputs(**inputs)
    res = bass_utils.run_bass_kernel_spmd(
        nc, in_maps, core_ids=list(range(N_CORES)))
    return _combine([res.results[k]["out"] for k in range(N_CORES)])


def _combine(outs):
    """Host-side gather: softmax over per-cluster logits + weighted sum."""
    logits = np.array([float(np.asarray(o)[0, 4]) for o in outs],
                      dtype=np.float64)
    h_path = np.stack([np.asarray(o)[:, [0, 2]].T.reshape(DHID)
                       for o in outs])
    w = np.exp(logits - logits.max())
    w /= w.sum()
    H = (w[:, None] * h_path.astype(np.float64)).sum(axis=0)
    return np.ascontiguousarray(H.reshape(1, DHID)).astype(np.float32)


# revision 25
# speedup vs baseline: 1.1944x; 1.0211x over previous
"""
DeepAttMISL segment-reduce kernel for Trainium2 (Bass/Tile), 8 NeuronCores.

Math (see reference):
  h        = relu(x @ W1.T + b1)                    x:[N,1024] -> h:[N,256]
  seg      = segment_sum(h, cluster_id, 8)          -> [8,256]
  h_clust  = seg / max(counts,1)
  h_path   = relu(h_clust @ Wf.T + bf)
  A        = softmax((tanh(h_path@Wa.T+ba) * sigmoid(h_path@Wb.T+bb)) @ Wc.T)
  H        = A @ h_path                             -> [1,256]

Sharding: BY CLUSTER, not by rows.  Core k receives ALL rows of cluster k
(host sorts rows by cluster_id), zero-padded to a fixed NPAD rows.  Each
core therefore owns its cluster's full segment sum locally and NO cross-core
collective is needed (the ncfw AllReduce costs 25-35us per op in this
runtime, plus a ~56us wake, and dominated the previous version's critical
path).  Each core runs the tiny gated-attention head for its own cluster and
outputs (logit_k, h_path_k); the host does the final 8-way softmax +
weighted sum as the gather/unshard step.

Main matmul is computed TRANSPOSED (W1 stationary, x moving, h.T in PSUM
[hid_half, rows]) so the segment sum falls out of ACT's accum_out: one
activation op per PSUM tile does bias + relu + sum-over-rows.  No segment
matmuls, no one-hot matrix.  Zero-pad rows contribute exactly relu(b1)
each; the host bakes -n_pad*relu(b1)/count into a per-core correction.

x is streamed as NBLK contiguous 1MiB DMAs (8KiB per partition per block)
on the sync ring - near line rate.  bf16 everywhere in the big matmul
(fp8 fails the 2e-2 gate: W1's quantization error is shared across
instances so it does not average out); fp32 head.  sigmoid(y) =
0.5*(1+tanh(y/2)) with the 0.5 folded into Wc so one ACT table set
(relu/tanh/exp) serves the whole kernel.
"""

import sys

if "/opt/trn_rl_repo" not in sys.path:
    sys.path.insert(0, "/opt/trn_rl_repo")

import numpy as np
import ml_dtypes

import concourse.bass as bass
import concourse.tile as tile
from concourse import bacc, mybir
from concourse import bass_utils

ALU = mybir.AluOpType

N_CORES = 8
N_TOTAL = 65536
DIN = 1024
DHID = 256
K_CL = 8
KC = DIN // 128                        # 8 contraction chunks of 128
BLK = 448                              # bulk rows per block (<=512 fp32 PSUM
                                       # bank; 448 cuts zero-padding to ~2%)
BLK0 = 224                             # first/last block halved: block 0
                                       # completes its DMA sooner (earlier PE
                                       # start), same total padding
WARMUP_MMS = 11                        # PE bridge: engine free ~8.2us, block0
                                       # ~12.5us; also >=3.4us busy for HAM

# wblob: head weights, float32r (PE-only consumers), per-partition fp32 elems
OFF_WFT = 0                            # Wf.T tiled   [2,256] -> 512
OFF_WAT = 512                          # Wa.T tiled   [2,256] -> 512
OFF_WBT = 1024                         # (Wb/2).T     [2,256] -> 512
OFF_WCR = 1536                         # (Wc/2) bcast [2,128] -> 256
OFF_BFR = 1792                         # bf as a row (bcast to all p) -> 256
OFF_BAR = 2048                         # ba row                       -> 256
OFF_BBR = 2304                         # bb/2 row                     -> 256
OFF_ONE = 2560                         # constant 1.0 x2 (rhs of bias MMs)
NWBLOB = 2562
# sblob: fp32 scalars for ACT bias / DVE (b1 tiled, padding fix, 1/count)
OFF_B1C = 0                            # b1 tiled [2]
OFF_CORR = 2                           # -invc*n_pad*relu(b1), dup pairs [4]
OFF_INVC = 6                           # 1/max(count,1) scalar [1]
NSBLOB = 7

BF16 = mybir.dt.bfloat16
F32 = mybir.dt.float32
AF = mybir.ActivationFunctionType

_CACHE = {}


def _build_nc(sizes):
    nblk = len(sizes)
    offs = [0]
    for s in sizes:
        offs.append(offs[-1] + s)
    tot = offs[-1]
    nc = bacc.Bacc("TRN2", target_bir_lowering=False, debug=False,
                   num_devices=N_CORES)

    xb = nc.dram_tensor("xb", [128, KC * tot], BF16,
                        kind="ExternalInput")
    w1t = nc.dram_tensor("w1t", [128, KC * DHID], BF16, kind="ExternalInput")
    wblob = nc.dram_tensor("wblob", [128, NWBLOB], mybir.dt.float32r,
                           kind="ExternalInput")
    sblob = nc.dram_tensor("sblob", [128, NSBLOB], F32, kind="ExternalInput")
    out = nc.dram_tensor("out", [128, 6], F32, kind="ExternalOutput")

    with tile.TileContext(nc) as tc:
        with tc.tile_pool(name="consts", bufs=1) as consts, \
             tc.tile_pool(name="xblk", bufs=1) as xblk, \
             tc.tile_pool(name="hps", bufs=4, space="PSUM") as hps, \
             tc.tile_pool(name="headps", bufs=2, space="PSUM") as headps, \
             tc.tile_pool(name="small", bufs=1) as small:

            # ---- PE warm-up bridge: keep HAM busy (and un-throttled by the
            # time real data arrives) from t~0 until block 0 lands (~13us).
            wz = consts.tile([128, BLK], BF16)
            nc.vector.memset(wz[:], 0.0)
            wps = hps.tile([128, BLK], F32, tag="main")
            for _ in range(WARMUP_MMS):
                nc.tensor.matmul(wps[:], wz[:, 0:128], wz[:],
                                 start=True, stop=True, skip_group_check=True)

            # ---- DMAs: ALL on the sync ring, in consumption order.  The
            # tiny sblob goes first purely to eat the ~1us cold-first-
            # descriptor cost on all 16 SDMA engines; then W1 (gates the
            # first MM), then the x blocks as back-to-back ~0.9MiB
            # transfers (7KiB/partition lines, full line rate), and the
            # head-weight blob LAST (needed only ~60us in; anywhere earlier
            # it steals SDMA bandwidth from block 0 and delays the whole
            # pipeline).
            F32R = mybir.dt.float32r
            w1t_sb = consts.tile([128, KC, DHID], BF16)
            w1v = w1t.ap().rearrange("p (k f) -> p k f", k=KC)
            # W1 in two pieces: the first eats the SDMA engines' cold
            # first-descriptor latency (~0.6-1us) while doing useful work
            nc.sync.dma_start(w1t_sb[:, 0:2], w1v[:, 0:2])
            nc.sync.dma_start(w1t_sb[:, 2:KC], w1v[:, 2:KC])

            xts = xblk.tile([128, KC * tot], BF16)
            sblob_sb = consts.tile([128, NSBLOB], F32)
            for b in range(nblk):
                nc.sync.dma_start(
                    xts[:, KC * offs[b]:KC * offs[b + 1]],
                    xb.ap()[:, KC * offs[b]:KC * offs[b + 1]])
                if b == 0:
                    nc.sync.dma_start(sblob_sb[:], sblob.ap())
            wblob_sb = consts.tile([128, NWBLOB], F32R)
            nc.sync.dma_start(wblob_sb[:], wblob.ap())

            # ---- main loop: per block, per hid-half: 8 accumulated MMs
            # (W1 chunk stationary, x moving, N=512), then one ACT op doing
            # bias + relu + accum_out (the segment sum over this block).
            segparts = small.tile([128, 2, nblk], F32)
            seghalf = small.tile([128, 2], F32)
            hsc = [small.tile([128, BLK], BF16, name=f"hsc{i}")
                   for i in range(3)]
            bhalf = nblk // 2
            for b in range(nblk):
                sz, off = sizes[b], offs[b]
                for j in range(2):
                    ps = hps.tile([128, sz], F32, tag="main",
                                  padded_shape=[128, 512])
                    for c in range(KC):
                        nc.tensor.matmul(
                            ps[:],
                            w1t_sb[:, c, j * 128:(j + 1) * 128],
                            xts[:, KC * off + c * sz:KC * off + (c + 1) * sz],
                            start=(c == 0), stop=(c == KC - 1),
                            skip_group_check=True)
                    nc.scalar.activation(
                        hsc[(2 * b + j) % 3][:, 0:sz], ps[:], AF.Relu,
                        bias=sblob_sb[:, OFF_B1C + j:OFF_B1C + j + 1],
                        accum_out=segparts[:, j, b:b + 1])
                if b == bhalf:
                    # fold the first half of the block partials early, off
                    # the critical tail (DVE is idle mid-loop)
                    for j in range(2):
                        nc.vector.reduce_sum(seghalf[:, j:j + 1],
                                             segparts[:, j, 0:bhalf + 1],
                                             axis=mybir.AxisListType.X)

            # ---- local segment sum -> cluster mean (with padding fix).
            # Columns come in duplicated pairs (cols 2j and 2j+1 equal):
            # the fp32r matmul ISA requires an even moving free dim, so the
            # whole head works on [128, 4] with N=2 GEMVs.
            seg4 = small.tile([128, 4], F32)
            for j in range(2):
                nc.vector.reduce_sum(seg4[:, 2 * j:2 * j + 1],
                                     segparts[:, j, bhalf + 1:],
                                     axis=mybir.AxisListType.X)
            for j in range(2):
                nc.vector.tensor_add(seg4[:, 2 * j:2 * j + 1],
                                     seg4[:, 2 * j:2 * j + 1],
                                     seghalf[:, j:j + 1])
                nc.vector.tensor_copy(seg4[:, 2 * j + 1:2 * j + 2],
                                      seg4[:, 2 * j:2 * j + 1])
            hc = small.tile([128, 4], F32R)
            nc.vector.tensor_scalar_mul(hc[:], seg4[:],
                                        sblob_sb[:, OFF_INVC:OFF_INVC + 1])
            nc.vector.tensor_add(hc[:], hc[:],
                                 sblob_sb[:, OFF_CORR:OFF_CORR + 4])

            # ---- gated-attention head for this core's cluster ----
            # fp32r matmuls (1-pass fp22, single LDW pass vs fp32's two of
            # each); bias folded into the accumulation group as a K=1
            # matmul against a constant-one rhs so one ACT handles both
            # hid-halves of a layer.
            def head_layer(w_off, b_off, rhs, func, name):
                o = small.tile([128, 4], F32R, name=name)
                ps = headps.tile([128, 4], F32, tag="head",
                                 padded_shape=[128, BLK])
                n = 0
                for j in range(2):
                    for i in range(2):
                        nc.tensor.matmul(
                            ps[:, 2 * j:2 * j + 2],
                            wblob_sb[:, w_off + i * 256 + j * 128:
                                     w_off + i * 256 + (j + 1) * 128],
                            rhs[:, 2 * i:2 * i + 2],
                            start=(n == 0), stop=False,
                            skip_group_check=True)
                        n += 1
                    nc.tensor.matmul(
                        ps[:, 2 * j:2 * j + 2],
                        wblob_sb[0:1, b_off + j * 128:
                                 b_off + (j + 1) * 128],
                        wblob_sb[0:1, OFF_ONE:OFF_ONE + 2],
                        start=False, stop=(j == 1), skip_group_check=True)
                nc.scalar.activation(o[:], ps[:], func)
                return o

            fps = headps.tile([128, 2], F32, tag="fill",
                              padded_shape=[128, 512])

            def pe_filler(n):
                # dummy f32r MMs with no data deps: they run while the next
                # layer waits on its ACT, keeping HAM from re-throttling
                for _ in range(n):
                    nc.tensor.matmul(
                        fps[:], wblob_sb[:, 0:128],
                        wblob_sb[:, OFF_ONE:OFF_ONE + 2],
                        start=True, stop=True, skip_group_check=True)

            pe_filler(4)
            hpT = head_layer(OFF_WFT, OFF_BFR, hc, AF.Relu, "hpT")
            pe_filler(3)
            aT = head_layer(OFF_WAT, OFF_BAR, hpT, AF.Tanh, "aT")
            tT = head_layer(OFF_WBT, OFF_BBR, hpT, AF.Tanh, "tT")
            pe_filler(3)
            # a*g = 0.5*a*(1+tanh(y/2)); the 0.5 lives in Wc/2
            ag = small.tile([128, 4], F32R)
            nc.vector.tensor_mul(ag[:], aT[:], tT[:])
            nc.vector.tensor_add(ag[:], ag[:], aT[:])

            # logit (replicated across partitions via broadcast Wc/2)
            lps = headps.tile([128, 2], F32, tag="head",
                              padded_shape=[128, BLK])
            for j in range(2):
                nc.tensor.matmul(
                    lps[:],
                    wblob_sb[:, OFF_WCR + j * 128:
                             OFF_WCR + (j + 1) * 128],
                    ag[:, 2 * j:2 * j + 2],
                    start=(j == 0), stop=(j == 1))

            # stream h_path out as soon as it's ready (overlaps the gate
            # matmuls); the logit follows in a second small DMA
            nc.sync.dma_start(out.ap()[:, 0:4].bitcast(F32R), hpT[:])
            lsb = small.tile([128, 2], F32)
            nc.vector.tensor_copy(lsb[:], lps[:])
            nc.sync.dma_start(out.ap()[:, 4:6], lsb[:])

    nc.compile()
    return nc


def _shard_plan(cluster_id):
    cid = np.asarray(cluster_id).astype(np.int64).reshape(N_TOTAL)
    counts = np.bincount(cid, minlength=K_CL).astype(np.int64)
    mx = int(counts.max())
    k = max(0, -(-(mx - 4 * BLK0) // BLK))             # ceil((mx-896)/448)
    # three small lead blocks: the DMA stream ramps slowly (one laggard SDMA
    # engine gates each block's completion), so early blocks complete sooner
    # and the PE pipeline starts ~3us earlier at identical total padding
    sizes = (BLK0,) * 3 + (BLK,) * k + (BLK0,)
    return cid, counts, sizes


def _prep_inputs(x_path, cluster_id, W1, b1, Wf, bf, Wa, ba, Wb, bb, Wc, bc):
    """Host-side sharding / marshalling. Returns (in_maps, sizes)."""
    cid, counts, sizes = _shard_plan(cluster_id)
    npad = sum(sizes)
    x = np.asarray(x_path, dtype=np.float32).reshape(N_TOTAL, DIN)
    xb16 = x.astype(ml_dtypes.bfloat16)

    W1 = np.asarray(W1, np.float32); b1 = np.asarray(b1, np.float32)
    Wf = np.asarray(Wf, np.float32); bf = np.asarray(bf, np.float32)
    Wa = np.asarray(Wa, np.float32); ba = np.asarray(ba, np.float32)
    Wb = np.asarray(Wb, np.float32); bb = np.asarray(bb, np.float32)
    Wc = np.asarray(Wc, np.float32)

    def tiled_T(M):  # [256,256] -> [128, 512]; [p, j*256+f] = M.T[j*128+p, f]
        return np.ascontiguousarray(
            M.T.reshape(2, 128, DHID).transpose(1, 0, 2)).reshape(128, 512)

    def tiled_v(v):  # [256] -> [128, 2]; [p, j] = v[j*128+p]
        return np.ascontiguousarray(v.reshape(2, 128).T)

    wblob = np.zeros((128, NWBLOB), np.float32)
    wblob[:, OFF_WFT:OFF_WFT + 512] = tiled_T(Wf)
    wblob[:, OFF_WAT:OFF_WAT + 512] = tiled_T(Wa)
    wblob[:, OFF_WBT:OFF_WBT + 512] = tiled_T(Wb * 0.5)
    wcr = np.broadcast_to((Wc.ravel() * 0.5).reshape(2, 128, 1),
                          (2, 128, 128)).transpose(1, 0, 2)
    wblob[:, OFF_WCR:OFF_WCR + 256] = wcr.reshape(128, 256)
    wblob[:, OFF_BFR:OFF_BFR + 256] = bf[None, :]
    wblob[:, OFF_BAR:OFF_BAR + 256] = ba[None, :]
    wblob[:, OFF_BBR:OFF_BBR + 256] = (bb * 0.5)[None, :]
    wblob[:, OFF_ONE:OFF_ONE + 2] = 1.0
    sblob_base = np.zeros((128, NSBLOB), np.float32)
    sblob_base[:, OFF_B1C:OFF_B1C + 2] = tiled_v(b1)

    # W1.T tiled: [p, c*256+m] = W1[m, c*128+p]
    w1tt = np.ascontiguousarray(
        W1.T.reshape(KC, 128, DHID).transpose(1, 0, 2)
    ).reshape(128, KC * DHID).astype(ml_dtypes.bfloat16)

    relu_b1 = np.maximum(b1, 0.0).astype(np.float32)

    in_maps = []
    for k in range(N_CORES):
        rows = np.nonzero(cid == k)[0]
        nk = len(rows)
        shard = np.zeros((npad, DIN), dtype=ml_dtypes.bfloat16)
        shard[:nk] = xb16[rows]
        # per block: [sz, 1024] -> [p, c, r]; concat along cols
        parts, off = [], 0
        for sz in sizes:
            parts.append(shard[off:off + sz].reshape(sz, KC, 128)
                         .transpose(2, 1, 0).reshape(128, KC * sz))
            off += sz
        xcore = np.ascontiguousarray(np.concatenate(parts, axis=1))

        invc = np.float32(1.0 / max(float(counts[k]), 1.0))
        n_pad = float(npad - nk)
        sblob_k = sblob_base.copy()
        corr = tiled_v((-invc * n_pad) * relu_b1)     # [128, 2]
        sblob_k[:, OFF_CORR:OFF_CORR + 4] = corr[:, [0, 0, 1, 1]]
        sblob_k[:, OFF_INVC] = invc
        in_maps.append({"xb": xcore, "w1t": w1tt, "wblob": wblob,
                        "sblob": sblob_k})
    return in_maps, sizes


def kernel(**inputs):
    _, _, sizes = _shard_plan(inputs["cluster_id"])
    key = ("nc", sizes)
    if key not in _CACHE:
        _CACHE[key] = _build_nc(sizes)
        _CACHE["nc"] = _CACHE[key]       # convenience handle for test.py
    nc = _CACHE[key]
    in_maps, _ = _prep_inputs(**inputs)
    res = bass_utils.run_bass_kernel_spmd(
        nc, in_maps, core_ids=list(range(N_CORES)))
    return _combine([res.results[k]["out"] for k in range(N_CORES)])


def _combine(outs):
    """Host-side gather: softmax over per-cluster logits + weighted sum."""
    logits = np.array([float(np.asarray(o)[0, 4]) for o in outs],
                      dtype=np.float64)
    h_path = np.stack([np.asarray(o)[:, [0, 2]].T.reshape(DHID)
                       for o in outs])
    w = np.exp(logits - logits.max())
    w /= w.sum()
    H = (w[:, None] * h_path.astype(np.float64)).sum(axis=0)
    return np.ascontiguousarray(H.reshape(1, DHID)).astype(np.float32)


# revision 27
# speedup vs baseline: 1.1949x; 1.0004x over previous
"""
DeepAttMISL segment-reduce kernel for Trainium2 (Bass/Tile), 8 NeuronCores.

Math (see reference):
  h        = relu(x @ W1.T + b1)                    x:[N,1024] -> h:[N,256]
  seg      = segment_sum(h, cluster_id, 8)          -> [8,256]
  h_clust  = seg / max(counts,1)
  h_path   = relu(h_clust @ Wf.T + bf)
  A        = softmax((tanh(h_path@Wa.T+ba) * sigmoid(h_path@Wb.T+bb)) @ Wc.T)
  H        = A @ h_path                             -> [1,256]

Sharding: BY CLUSTER, not by rows.  Core k receives ALL rows of cluster k
(host sorts rows by cluster_id), zero-padded to a fixed NPAD rows.  Each
core therefore owns its cluster's full segment sum locally and NO cross-core
collective is needed (the ncfw AllReduce costs 25-35us per op in this
runtime, plus a ~56us wake, and dominated the previous version's critical
path).  Each core runs the tiny gated-attention head for its own cluster and
outputs (logit_k, h_path_k); the host does the final 8-way softmax +
weighted sum as the gather/unshard step.

Main matmul is computed TRANSPOSED (W1 stationary, x moving, h.T in PSUM
[hid_half, rows]) so the segment sum falls out of ACT's accum_out: one
activation op per PSUM tile does bias + relu + sum-over-rows.  No segment
matmuls, no one-hot matrix.  Zero-pad rows contribute exactly relu(b1)
each; the host bakes -n_pad*relu(b1)/count into a per-core correction.

x is streamed as per-block contiguous DMAs on the sync ring at line rate
(~425GB/s), with small lead blocks to beat the DMA ramp (per-transfer cold
descriptor + laggard-engine completion skew) and a small tail block to trim
padding.  All DMAs ride the sync ring in consumption order (W1 split so its
first piece eats the cold descriptor; head weights last).  bf16 everywhere
in the big matmul (fp8 fails the 2e-2 gate: W1's quantization error is
shared across instances so it does not average out); float32r (fp22) head
with duplicated-pair columns (the fp32r ISA needs an even moving free dim)
and biases folded in as K=1 matmuls.  PE warm-up bridges the ~13us DMA
lead-in and filler matmuls keep HAM from re-throttling across the head.
sigmoid(y) = 0.5*(1+tanh(y/2)) with the 0.5 folded into Wc so one ACT
table set (relu/tanh/exp) serves the whole kernel.
"""

import sys

if "/opt/trn_rl_repo" not in sys.path:
    sys.path.insert(0, "/opt/trn_rl_repo")

import numpy as np
import ml_dtypes

import concourse.bass as bass
import concourse.tile as tile
from concourse import bacc, mybir
from concourse import bass_utils

ALU = mybir.AluOpType

N_CORES = 8
N_TOTAL = 65536
DIN = 1024
DHID = 256
K_CL = 8
KC = DIN // 128                        # 8 contraction chunks of 128
BLK = 448                              # bulk rows per block (<=512 fp32 PSUM
                                       # bank; 448 cuts zero-padding to ~2%)
BLK0 = 224                             # first/last block halved: block 0
                                       # completes its DMA sooner (earlier PE
                                       # start), same total padding
WARMUP_MMS = 15                        # PE bridge: engine free ~7.4us, block0
                                       # ~13us; also >=3.4us busy for HAM

# wblob: head weights, float32r (PE-only consumers), per-partition fp32 elems
OFF_WFT = 0                            # Wf.T tiled   [2,256] -> 512
OFF_WAT = 512                          # Wa.T tiled   [2,256] -> 512
OFF_WBT = 1024                         # (Wb/2).T     [2,256] -> 512
OFF_WCR = 1536                         # (Wc/2) bcast [2,128] -> 256
OFF_BFR = 1792                         # bf as a row (bcast to all p) -> 256
OFF_BAR = 2048                         # ba row                       -> 256
OFF_BBR = 2304                         # bb/2 row                     -> 256
OFF_ONE = 2560                         # constant 1.0 x2 (rhs of bias MMs)
NWBLOB = 2562
# sblob: fp32 scalars for ACT bias / DVE (b1 tiled, padding fix, 1/count)
OFF_B1C = 0                            # b1 tiled [2]
OFF_CORR = 2                           # -invc*n_pad*relu(b1), dup pairs [4]
OFF_INVC = 6                           # 1/max(count,1) scalar [1]
NSBLOB = 7

BF16 = mybir.dt.bfloat16
F32 = mybir.dt.float32
AF = mybir.ActivationFunctionType

_CACHE = {}


def _build_nc(sizes):
    nblk = len(sizes)
    offs = [0]
    for s in sizes:
        offs.append(offs[-1] + s)
    tot = offs[-1]
    nc = bacc.Bacc("TRN2", target_bir_lowering=False, debug=False,
                   num_devices=N_CORES)

    xb = nc.dram_tensor("xb", [128, KC * tot], BF16,
                        kind="ExternalInput")
    w1t = nc.dram_tensor("w1t", [128, KC * DHID], BF16, kind="ExternalInput")
    wblob = nc.dram_tensor("wblob", [128, NWBLOB], mybir.dt.float32r,
                           kind="ExternalInput")
    sblob = nc.dram_tensor("sblob", [128, NSBLOB], F32, kind="ExternalInput")
    out = nc.dram_tensor("out", [128, 6], F32, kind="ExternalOutput")

    with tile.TileContext(nc) as tc:
        with tc.tile_pool(name="consts", bufs=1) as consts, \
             tc.tile_pool(name="xblk", bufs=1) as xblk, \
             tc.tile_pool(name="hps", bufs=4, space="PSUM") as hps, \
             tc.tile_pool(name="headps", bufs=2, space="PSUM") as headps, \
             tc.tile_pool(name="small", bufs=1) as small:

            # ---- PE warm-up bridge: keep HAM busy (and un-throttled by the
            # time real data arrives) from t~0 until block 0 lands (~13us).
            wz = consts.tile([128, BLK], BF16)
            nc.vector.memset(wz[:], 0.0)
            wps = hps.tile([128, BLK], F32, tag="main")
            for _ in range(WARMUP_MMS):
                nc.tensor.matmul(wps[:], wz[:, 0:128], wz[:],
                                 start=True, stop=True, skip_group_check=True)

            # ---- DMAs: ALL on the sync ring, in consumption order.  The
            # tiny sblob goes first purely to eat the ~1us cold-first-
            # descriptor cost on all 16 SDMA engines; then W1 (gates the
            # first MM), then the x blocks as back-to-back ~0.9MiB
            # transfers (7KiB/partition lines, full line rate), and the
            # head-weight blob LAST (needed only ~60us in; anywhere earlier
            # it steals SDMA bandwidth from block 0 and delays the whole
            # pipeline).
            F32R = mybir.dt.float32r
            w1t_sb = consts.tile([128, KC, DHID], BF16)
            w1v = w1t.ap().rearrange("p (k f) -> p k f", k=KC)
            # W1 in two pieces: the first eats the SDMA engines' cold
            # first-descriptor latency (~0.6-1us) while doing useful work
            nc.sync.dma_start(w1t_sb[:, 0:2], w1v[:, 0:2])
            nc.sync.dma_start(w1t_sb[:, 2:KC], w1v[:, 2:KC])

            xts = xblk.tile([128, KC * tot], BF16)
            sblob_sb = consts.tile([128, NSBLOB], F32)
            for b in range(nblk):
                nc.sync.dma_start(
                    xts[:, KC * offs[b]:KC * offs[b + 1]],
                    xb.ap()[:, KC * offs[b]:KC * offs[b + 1]])
                if b == 0:
                    nc.sync.dma_start(sblob_sb[:], sblob.ap())
            wblob_sb = consts.tile([128, NWBLOB], F32R)
            nc.sync.dma_start(wblob_sb[:], wblob.ap())

            # ---- main loop: per block, per hid-half: 8 accumulated MMs
            # (W1 chunk stationary, x moving, N=512), then one ACT op doing
            # bias + relu + accum_out (the segment sum over this block).
            segparts = small.tile([128, 2, nblk], F32)
            seghalf = small.tile([128, 2], F32)
            hsc = [small.tile([128, BLK], BF16, name=f"hsc{i}")
                   for i in range(3)]
            bhalf = nblk // 2
            for b in range(nblk):
                sz, off = sizes[b], offs[b]
                for j in range(2):
                    ps = hps.tile([128, sz], F32, tag="main",
                                  padded_shape=[128, 512])
                    for c in range(KC):
                        nc.tensor.matmul(
                            ps[:],
                            w1t_sb[:, c, j * 128:(j + 1) * 128],
                            xts[:, KC * off + c * sz:KC * off + (c + 1) * sz],
                            start=(c == 0), stop=(c == KC - 1),
                            skip_group_check=True)
                    nc.scalar.activation(
                        hsc[(2 * b + j) % 3][:, 0:sz], ps[:], AF.Relu,
                        bias=sblob_sb[:, OFF_B1C + j:OFF_B1C + j + 1],
                        accum_out=segparts[:, j, b:b + 1])
                if b == bhalf:
                    # fold the first half of the block partials early, off
                    # the critical tail (DVE is idle mid-loop)
                    for j in range(2):
                        nc.vector.reduce_sum(seghalf[:, j:j + 1],
                                             segparts[:, j, 0:bhalf + 1],
                                             axis=mybir.AxisListType.X)

            # ---- local segment sum -> cluster mean (with padding fix).
            # Columns come in duplicated pairs (cols 2j and 2j+1 equal):
            # the fp32r matmul ISA requires an even moving free dim, so the
            # whole head works on [128, 4] with N=2 GEMVs.
            seg4 = small.tile([128, 4], F32)
            for j in range(2):
                nc.vector.reduce_sum(seg4[:, 2 * j:2 * j + 1],
                                     segparts[:, j, bhalf + 1:],
                                     axis=mybir.AxisListType.X)
            for j in range(2):
                nc.vector.tensor_add(seg4[:, 2 * j:2 * j + 1],
                                     seg4[:, 2 * j:2 * j + 1],
                                     seghalf[:, j:j + 1])
                nc.vector.tensor_copy(seg4[:, 2 * j + 1:2 * j + 2],
                                      seg4[:, 2 * j:2 * j + 1])
            hc = small.tile([128, 4], F32R)
            nc.vector.tensor_scalar_mul(hc[:], seg4[:],
                                        sblob_sb[:, OFF_INVC:OFF_INVC + 1])
            nc.vector.tensor_add(hc[:], hc[:],
                                 sblob_sb[:, OFF_CORR:OFF_CORR + 4])

            # ---- gated-attention head for this core's cluster ----
            # fp32r matmuls (1-pass fp22, single LDW pass vs fp32's two of
            # each); bias folded into the accumulation group as a K=1
            # matmul against a constant-one rhs so one ACT handles both
            # hid-halves of a layer.
            def head_layer(w_off, b_off, rhs, func, name):
                o = small.tile([128, 4], F32R, name=name)
                ps = headps.tile([128, 4], F32, tag="head",
                                 padded_shape=[128, BLK])
                n = 0
                for j in range(2):
                    for i in range(2):
                        nc.tensor.matmul(
                            ps[:, 2 * j:2 * j + 2],
                            wblob_sb[:, w_off + i * 256 + j * 128:
                                     w_off + i * 256 + (j + 1) * 128],
                            rhs[:, 2 * i:2 * i + 2],
                            start=(n == 0), stop=False,
                            skip_group_check=True)
                        n += 1
                    nc.tensor.matmul(
                        ps[:, 2 * j:2 * j + 2],
                        wblob_sb[0:1, b_off + j * 128:
                                 b_off + (j + 1) * 128],
                        wblob_sb[0:1, OFF_ONE:OFF_ONE + 2],
                        start=False, stop=(j == 1), skip_group_check=True)
                nc.scalar.activation(o[:], ps[:], func)
                return o

            fps = headps.tile([128, 2], F32, tag="fill",
                              padded_shape=[128, 512])

            def pe_filler(n):
                # dummy f32r MMs with no data deps: they run while the next
                # layer waits on its ACT, keeping HAM from re-throttling
                for _ in range(n):
                    nc.tensor.matmul(
                        fps[:], wblob_sb[:, 0:128],
                        wblob_sb[:, OFF_ONE:OFF_ONE + 2],
                        start=True, stop=True, skip_group_check=True)

            pe_filler(10)
            hpT = head_layer(OFF_WFT, OFF_BFR, hc, AF.Relu, "hpT")
            pe_filler(3)
            aT = head_layer(OFF_WAT, OFF_BAR, hpT, AF.Tanh, "aT")
            tT = head_layer(OFF_WBT, OFF_BBR, hpT, AF.Tanh, "tT")
            pe_filler(3)
            # a*g = 0.5*a*(1+tanh(y/2)); the 0.5 lives in Wc/2
            ag = small.tile([128, 4], F32R)
            nc.vector.tensor_mul(ag[:], aT[:], tT[:])
            nc.vector.tensor_add(ag[:], ag[:], aT[:])

            # logit (replicated across partitions via broadcast Wc/2)
            lps = headps.tile([128, 2], F32, tag="head",
                              padded_shape=[128, BLK])
            for j in range(2):
                nc.tensor.matmul(
                    lps[:],
                    wblob_sb[:, OFF_WCR + j * 128:
                             OFF_WCR + (j + 1) * 128],
                    ag[:, 2 * j:2 * j + 2],
                    start=(j == 0), stop=(j == 1))

            # stream h_path out as soon as it's ready (overlaps the gate
            # matmuls); the logit follows in a second small DMA
            nc.sync.dma_start(out.ap()[:, 0:4].bitcast(F32R), hpT[:])
            lsb = small.tile([128, 2], F32)
            nc.vector.tensor_copy(lsb[:], lps[:])
            nc.sync.dma_start(out.ap()[:, 4:6], lsb[:])

    nc.compile()
    return nc


def _shard_plan(cluster_id):
    cid = np.asarray(cluster_id).astype(np.int64).reshape(N_TOTAL)
    counts = np.bincount(cid, minlength=K_CL).astype(np.int64)
    mx = int(counts.max())
    # small lead blocks: the DMA stream ramps slowly (per-transfer cold
    # descriptor + one laggard SDMA engine gates each completion), so early
    # blocks complete sooner and the PE pipeline starts ~2us earlier; a
    # small tail block also trims zero-padding to <=447 rows of quantum 112
    lead = (112, 112, 224, 224)
    k = max(0, -(-(mx - sum(lead) - 112) // BLK))
    sizes = lead + (BLK,) * k + (112,)
    return cid, counts, sizes


def _prep_inputs(x_path, cluster_id, W1, b1, Wf, bf, Wa, ba, Wb, bb, Wc, bc):
    """Host-side sharding / marshalling. Returns (in_maps, sizes)."""
    cid, counts, sizes = _shard_plan(cluster_id)
    npad = sum(sizes)
    x = np.asarray(x_path, dtype=np.float32).reshape(N_TOTAL, DIN)
    xb16 = x.astype(ml_dtypes.bfloat16)

    W1 = np.asarray(W1, np.float32); b1 = np.asarray(b1, np.float32)
    Wf = np.asarray(Wf, np.float32); bf = np.asarray(bf, np.float32)
    Wa = np.asarray(Wa, np.float32); ba = np.asarray(ba, np.float32)
    Wb = np.asarray(Wb, np.float32); bb = np.asarray(bb, np.float32)
    Wc = np.asarray(Wc, np.float32)

    def tiled_T(M):  # [256,256] -> [128, 512]; [p, j*256+f] = M.T[j*128+p, f]
        return np.ascontiguousarray(
            M.T.reshape(2, 128, DHID).transpose(1, 0, 2)).reshape(128, 512)

    def tiled_v(v):  # [256] -> [128, 2]; [p, j] = v[j*128+p]
        return np.ascontiguousarray(v.reshape(2, 128).T)

    wblob = np.zeros((128, NWBLOB), np.float32)
    wblob[:, OFF_WFT:OFF_WFT + 512] = tiled_T(Wf)
    wblob[:, OFF_WAT:OFF_WAT + 512] = tiled_T(Wa)
    wblob[:, OFF_WBT:OFF_WBT + 512] = tiled_T(Wb * 0.5)
    wcr = np.broadcast_to((Wc.ravel() * 0.5).reshape(2, 128, 1),
                          (2, 128, 128)).transpose(1, 0, 2)
    wblob[:, OFF_WCR:OFF_WCR + 256] = wcr.reshape(128, 256)
    wblob[:, OFF_BFR:OFF_BFR + 256] = bf[None, :]
    wblob[:, OFF_BAR:OFF_BAR + 256] = ba[None, :]
    wblob[:, OFF_BBR:OFF_BBR + 256] = (bb * 0.5)[None, :]
    wblob[:, OFF_ONE:OFF_ONE + 2] = 1.0
    sblob_base = np.zeros((128, NSBLOB), np.float32)
    sblob_base[:, OFF_B1C:OFF_B1C + 2] = tiled_v(b1)

    # W1.T tiled: [p, c*256+m] = W1[m, c*128+p]
    w1tt = np.ascontiguousarray(
        W1.T.reshape(KC, 128, DHID).transpose(1, 0, 2)
    ).reshape(128, KC * DHID).astype(ml_dtypes.bfloat16)

    relu_b1 = np.maximum(b1, 0.0).astype(np.float32)

    in_maps = []
    for k in range(N_CORES):
        rows = np.nonzero(cid == k)[0]
        nk = len(rows)
        shard = np.zeros((npad, DIN), dtype=ml_dtypes.bfloat16)
        shard[:nk] = xb16[rows]
        # per block: [sz, 1024] -> [p, c, r]; concat along cols
        parts, off = [], 0
        for sz in sizes:
            parts.append(shard[off:off + sz].reshape(sz, KC, 128)
                         .transpose(2, 1, 0).reshape(128, KC * sz))
            off += sz
        xcore = np.ascontiguousarray(np.concatenate(parts, axis=1))

        invc = np.float32(1.0 / max(float(counts[k]), 1.0))
        n_pad = float(npad - nk)
        sblob_k = sblob_base.copy()
        corr = tiled_v((-invc * n_pad) * relu_b1)     # [128, 2]
        sblob_k[:, OFF_CORR:OFF_CORR + 4] = corr[:, [0, 0, 1, 1]]
        sblob_k[:, OFF_INVC] = invc
        in_maps.append({"xb": xcore, "w1t": w1tt, "wblob": wblob,
                        "sblob": sblob_k})
    return in_maps, sizes


def kernel(**inputs):
    _, _, sizes = _shard_plan(inputs["cluster_id"])
    key = ("nc", sizes)
    if key not in _CACHE:
        _CACHE[key] = _build_nc(sizes)
        _CACHE["nc"] = _CACHE[key]       # convenience handle for test.py
    nc = _CACHE[key]
    in_maps, _ = _prep_inputs(**inputs)
    res = bass_utils.run_bass_kernel_spmd(
        nc, in_maps, core_ids=list(range(N_CORES)))
    return _combine([res.results[k]["out"] for k in range(N_CORES)])


def _combine(outs):
    """Host-side gather: softmax over per-cluster logits + weighted sum."""
    logits = np.array([float(np.asarray(o)[0, 4]) for o in outs],
                      dtype=np.float64)
    h_path = np.stack([np.asarray(o)[:, [0, 2]].T.reshape(DHID)
                       for o in outs])
    w = np.exp(logits - logits.max())
    w /= w.sum()
    H = (w[:, None] * h_path.astype(np.float64)).sum(axis=0)
    return np.ascontiguousarray(H.reshape(1, DHID)).astype(np.float32)


# revision 28
# speedup vs baseline: 1.2011x; 1.0052x over previous
"""
DeepAttMISL segment-reduce kernel for Trainium2 (Bass/Tile), 8 NeuronCores.

Math (see reference):
  h        = relu(x @ W1.T + b1)                    x:[N,1024] -> h:[N,256]
  seg      = segment_sum(h, cluster_id, 8)          -> [8,256]
  h_clust  = seg / max(counts,1)
  h_path   = relu(h_clust @ Wf.T + bf)
  A        = softmax((tanh(h_path@Wa.T+ba) * sigmoid(h_path@Wb.T+bb)) @ Wc.T)
  H        = A @ h_path                             -> [1,256]

Sharding: BY CLUSTER, not by rows.  Core k receives ALL rows of cluster k
(host sorts rows by cluster_id), zero-padded to a fixed NPAD rows.  Each
core therefore owns its cluster's full segment sum locally and NO cross-core
collective is needed (the ncfw AllReduce costs 25-35us per op in this
runtime, plus a ~56us wake, and dominated the previous version's critical
path).  Each core runs the tiny gated-attention head for its own cluster and
outputs (logit_k, h_path_k); the host does the final 8-way softmax +
weighted sum as the gather/unshard step.

Main matmul is computed TRANSPOSED (W1 stationary, x moving, h.T in PSUM
[hid_half, rows]) so the segment sum falls out of ACT's accum_out: one
activation op per PSUM tile does bias + relu + sum-over-rows.  No segment
matmuls, no one-hot matrix.  Zero-pad rows contribute exactly relu(b1)
each; the host bakes -n_pad*relu(b1)/count into a per-core correction.

x is streamed as per-block contiguous DMAs on the sync ring at line rate
(~425GB/s), with small lead blocks to beat the DMA ramp (per-transfer cold
descriptor + laggard-engine completion skew) and a small tail block to trim
padding.  All DMAs ride the sync ring in consumption order (W1 split so its
first piece eats the cold descriptor; head weights last).  bf16 everywhere
in the big matmul (fp8 fails the 2e-2 gate: W1's quantization error is
shared across instances so it does not average out); float32r (fp22) head
with duplicated-pair columns (the fp32r ISA needs an even moving free dim)
and biases folded in as K=1 matmuls.  PE warm-up bridges the ~13us DMA
lead-in and filler matmuls keep HAM from re-throttling across the head.
sigmoid(y) = 0.5*(1+tanh(y/2)) with the 0.5 folded into Wc so one ACT
table set (relu/tanh/exp) serves the whole kernel.
"""

import sys

if "/opt/trn_rl_repo" not in sys.path:
    sys.path.insert(0, "/opt/trn_rl_repo")

import numpy as np
import ml_dtypes

import concourse.bass as bass
import concourse.tile as tile
from concourse import bacc, mybir
from concourse import bass_utils

ALU = mybir.AluOpType

N_CORES = 8
N_TOTAL = 65536
DIN = 1024
DHID = 256
K_CL = 8
KC = DIN // 128                        # 8 contraction chunks of 128
BLK = 448                              # bulk rows per block (<=512 fp32 PSUM
                                       # bank; 448 cuts zero-padding to ~2%)
BLK0 = 224                             # first/last block halved: block 0
                                       # completes its DMA sooner (earlier PE
                                       # start), same total padding
WARMUP_MMS = 15                        # PE bridge: engine free ~7.4us, block0
                                       # ~13us; also >=3.4us busy for HAM

# wblob: head weights, float32r (PE-only consumers), per-partition fp32 elems
OFF_WFT = 0                            # Wf.T tiled   [2,256] -> 512
OFF_WAT = 512                          # Wa.T tiled   [2,256] -> 512
OFF_WBT = 1024                         # (Wb/2).T     [2,256] -> 512
OFF_WCR = 1536                         # (Wc/2) bcast [2,128] -> 256
OFF_BFR = 1792                         # bf as a row (bcast to all p) -> 256
OFF_BAR = 2048                         # ba row                       -> 256
OFF_BBR = 2304                         # bb/2 row                     -> 256
OFF_ONE = 2560                         # constant 1.0 x2 (rhs of bias MMs)
NWBLOB = 2562
# sblob: fp32 scalars for ACT bias / DVE (b1 tiled, padding fix, 1/count)
OFF_B1C = 0                            # b1 tiled [2]
OFF_CORR = 2                           # -invc*n_pad*relu(b1), dup pairs [4]
OFF_INVC = 6                           # 1/max(count,1) scalar [1]
NSBLOB = 7

BF16 = mybir.dt.bfloat16
F32 = mybir.dt.float32
AF = mybir.ActivationFunctionType

_CACHE = {}


def _build_nc(sizes):
    nblk = len(sizes)
    offs = [0]
    for s in sizes:
        offs.append(offs[-1] + s)
    tot = offs[-1]
    nc = bacc.Bacc("TRN2", target_bir_lowering=False, debug=False,
                   num_devices=N_CORES)

    xb = nc.dram_tensor("xb", [128, KC * tot], BF16,
                        kind="ExternalInput")
    w1t = nc.dram_tensor("w1t", [128, KC * DHID], BF16, kind="ExternalInput")
    wblob = nc.dram_tensor("wblob", [128, NWBLOB], mybir.dt.float32r,
                           kind="ExternalInput")
    sblob = nc.dram_tensor("sblob", [128, NSBLOB], F32, kind="ExternalInput")
    out = nc.dram_tensor("out", [128, 6], F32, kind="ExternalOutput")

    with tile.TileContext(nc) as tc:
        with tc.tile_pool(name="consts", bufs=1) as consts, \
             tc.tile_pool(name="xblk", bufs=1) as xblk, \
             tc.tile_pool(name="hps", bufs=4, space="PSUM") as hps, \
             tc.tile_pool(name="headps", bufs=2, space="PSUM") as headps, \
             tc.tile_pool(name="small", bufs=1) as small:

            # ---- PE warm-up bridge: keep HAM busy (and un-throttled by the
            # time real data arrives) from t~0 until block 0 lands (~13us).
            wz = consts.tile([128, BLK], BF16)
            nc.vector.memset(wz[:], 0.0)
            wps = hps.tile([128, BLK], F32, tag="main")
            for _ in range(WARMUP_MMS):
                nc.tensor.matmul(wps[:], wz[:, 0:128], wz[:],
                                 start=True, stop=True, skip_group_check=True)

            # ---- DMAs: ALL on the sync ring, in consumption order.  The
            # tiny sblob goes first purely to eat the ~1us cold-first-
            # descriptor cost on all 16 SDMA engines; then W1 (gates the
            # first MM), then the x blocks as back-to-back ~0.9MiB
            # transfers (7KiB/partition lines, full line rate), and the
            # head-weight blob LAST (needed only ~60us in; anywhere earlier
            # it steals SDMA bandwidth from block 0 and delays the whole
            # pipeline).
            F32R = mybir.dt.float32r
            w1t_sb = consts.tile([128, KC, DHID], BF16)
            # W1 + biases via SWDGE (gpsimd): that queue clears its preamble
            # ~4us before the sync queue does, so these land before x block 0
            nc.gpsimd.dma_start(w1t_sb[:], w1t.ap().rearrange(
                "p (k f) -> p k f", k=KC))
            sblob_sb = consts.tile([128, NSBLOB], F32)
            nc.gpsimd.dma_start(sblob_sb[:], sblob.ap())

            xts = xblk.tile([128, KC * tot], BF16)
            for b in range(nblk):
                nc.sync.dma_start(
                    xts[:, KC * offs[b]:KC * offs[b + 1]],
                    xb.ap()[:, KC * offs[b]:KC * offs[b + 1]])
            wblob_sb = consts.tile([128, NWBLOB], F32R)
            nc.sync.dma_start(wblob_sb[:], wblob.ap())

            # ---- main loop: per block, per hid-half: 8 accumulated MMs
            # (W1 chunk stationary, x moving, N=512), then one ACT op doing
            # bias + relu + accum_out (the segment sum over this block).
            segparts = small.tile([128, 2, nblk], F32)
            seghalf = small.tile([128, 2], F32)
            hsc = [small.tile([128, BLK], BF16, name=f"hsc{i}")
                   for i in range(3)]
            bhalf = nblk // 2
            for b in range(nblk):
                sz, off = sizes[b], offs[b]
                for j in range(2):
                    ps = hps.tile([128, sz], F32, tag="main",
                                  padded_shape=[128, 512])
                    for c in range(KC):
                        nc.tensor.matmul(
                            ps[:],
                            w1t_sb[:, c, j * 128:(j + 1) * 128],
                            xts[:, KC * off + c * sz:KC * off + (c + 1) * sz],
                            start=(c == 0), stop=(c == KC - 1),
                            skip_group_check=True)
                    nc.scalar.activation(
                        hsc[(2 * b + j) % 3][:, 0:sz], ps[:], AF.Relu,
                        bias=sblob_sb[:, OFF_B1C + j:OFF_B1C + j + 1],
                        accum_out=segparts[:, j, b:b + 1])
                if b == bhalf:
                    # fold the first half of the block partials early, off
                    # the critical tail (DVE is idle mid-loop)
                    for j in range(2):
                        nc.vector.reduce_sum(seghalf[:, j:j + 1],
                                             segparts[:, j, 0:bhalf + 1],
                                             axis=mybir.AxisListType.X)

            # ---- local segment sum -> cluster mean (with padding fix).
            # Columns come in duplicated pairs (cols 2j and 2j+1 equal):
            # the fp32r matmul ISA requires an even moving free dim, so the
            # whole head works on [128, 4] with N=2 GEMVs.
            seg4 = small.tile([128, 4], F32)
            for j in range(2):
                nc.vector.reduce_sum(seg4[:, 2 * j:2 * j + 1],
                                     segparts[:, j, bhalf + 1:],
                                     axis=mybir.AxisListType.X)
            for j in range(2):
                nc.vector.tensor_add(seg4[:, 2 * j:2 * j + 1],
                                     seg4[:, 2 * j:2 * j + 1],
                                     seghalf[:, j:j + 1])
                nc.vector.tensor_copy(seg4[:, 2 * j + 1:2 * j + 2],
                                      seg4[:, 2 * j:2 * j + 1])
            hc = small.tile([128, 4], F32R)
            nc.vector.tensor_scalar_mul(hc[:], seg4[:],
                                        sblob_sb[:, OFF_INVC:OFF_INVC + 1])
            nc.vector.tensor_add(hc[:], hc[:],
                                 sblob_sb[:, OFF_CORR:OFF_CORR + 4])

            # ---- gated-attention head for this core's cluster ----
            # fp32r matmuls (1-pass fp22, single LDW pass vs fp32's two of
            # each); bias folded into the accumulation group as a K=1
            # matmul against a constant-one rhs so one ACT handles both
            # hid-halves of a layer.
            def head_layer(w_off, b_off, rhs, func, name):
                o = small.tile([128, 4], F32R, name=name)
                ps = headps.tile([128, 4], F32, tag="head",
                                 padded_shape=[128, BLK])
                n = 0
                for j in range(2):
                    for i in range(2):
                        nc.tensor.matmul(
                            ps[:, 2 * j:2 * j + 2],
                            wblob_sb[:, w_off + i * 256 + j * 128:
                                     w_off + i * 256 + (j + 1) * 128],
                            rhs[:, 2 * i:2 * i + 2],
                            start=(n == 0), stop=False,
                            skip_group_check=True)
                        n += 1
                    nc.tensor.matmul(
                        ps[:, 2 * j:2 * j + 2],
                        wblob_sb[0:1, b_off + j * 128:
                                 b_off + (j + 1) * 128],
                        wblob_sb[0:1, OFF_ONE:OFF_ONE + 2],
                        start=False, stop=(j == 1), skip_group_check=True)
                nc.scalar.activation(o[:], ps[:], func)
                return o

            fps = headps.tile([128, 2], F32, tag="fill",
                              padded_shape=[128, 512])

            def pe_filler(n):
                # dummy f32r MMs with no data deps: they run while the next
                # layer waits on its ACT, keeping HAM from re-throttling
                for _ in range(n):
                    nc.tensor.matmul(
                        fps[:], wblob_sb[:, 0:128],
                        wblob_sb[:, OFF_ONE:OFF_ONE + 2],
                        start=True, stop=True, skip_group_check=True)

            pe_filler(12)
            hpT = head_layer(OFF_WFT, OFF_BFR, hc, AF.Relu, "hpT")
            pe_filler(6)
            aT = head_layer(OFF_WAT, OFF_BAR, hpT, AF.Tanh, "aT")
            tT = head_layer(OFF_WBT, OFF_BBR, hpT, AF.Tanh, "tT")
            pe_filler(6)
            # a*g = 0.5*a*(1+tanh(y/2)); the 0.5 lives in Wc/2
            ag = small.tile([128, 4], F32R)
            nc.vector.tensor_mul(ag[:], aT[:], tT[:])
            nc.vector.tensor_add(ag[:], ag[:], aT[:])

            # logit (replicated across partitions via broadcast Wc/2)
            lps = headps.tile([128, 2], F32, tag="head",
                              padded_shape=[128, BLK])
            for j in range(2):
                nc.tensor.matmul(
                    lps[:],
                    wblob_sb[:, OFF_WCR + j * 128:
                             OFF_WCR + (j + 1) * 128],
                    ag[:, 2 * j:2 * j + 2],
                    start=(j == 0), stop=(j == 1))

            # stream h_path out as soon as it's ready (overlaps the gate
            # matmuls); the logit follows in a second small DMA
            nc.sync.dma_start(out.ap()[:, 0:4].bitcast(F32R), hpT[:])
            lsb = small.tile([128, 2], F32)
            nc.vector.tensor_copy(lsb[:], lps[:])
            nc.sync.dma_start(out.ap()[:, 4:6], lsb[:])

    nc.compile()
    return nc


def _shard_plan(cluster_id):
    cid = np.asarray(cluster_id).astype(np.int64).reshape(N_TOTAL)
    counts = np.bincount(cid, minlength=K_CL).astype(np.int64)
    mx = int(counts.max())
    # small lead blocks: the DMA stream ramps slowly (per-transfer cold
    # descriptor + one laggard SDMA engine gates each completion), so early
    # blocks complete sooner and the PE pipeline starts ~2us earlier; a
    # small tail block also trims zero-padding to <=447 rows of quantum 112
    lead = (112, 112, 224, 224, 224, 224)
    k = max(0, -(-(mx - sum(lead) - 112) // BLK))
    sizes = lead + (BLK,) * k + (112,)
    return cid, counts, sizes


def _prep_inputs(x_path, cluster_id, W1, b1, Wf, bf, Wa, ba, Wb, bb, Wc, bc):
    """Host-side sharding / marshalling. Returns (in_maps, sizes)."""
    cid, counts, sizes = _shard_plan(cluster_id)
    npad = sum(sizes)
    x = np.asarray(x_path, dtype=np.float32).reshape(N_TOTAL, DIN)
    xb16 = x.astype(ml_dtypes.bfloat16)

    W1 = np.asarray(W1, np.float32); b1 = np.asarray(b1, np.float32)
    Wf = np.asarray(Wf, np.float32); bf = np.asarray(bf, np.float32)
    Wa = np.asarray(Wa, np.float32); ba = np.asarray(ba, np.float32)
    Wb = np.asarray(Wb, np.float32); bb = np.asarray(bb, np.float32)
    Wc = np.asarray(Wc, np.float32)

    def tiled_T(M):  # [256,256] -> [128, 512]; [p, j*256+f] = M.T[j*128+p, f]
        return np.ascontiguousarray(
            M.T.reshape(2, 128, DHID).transpose(1, 0, 2)).reshape(128, 512)

    def tiled_v(v):  # [256] -> [128, 2]; [p, j] = v[j*128+p]
        return np.ascontiguousarray(v.reshape(2, 128).T)

    wblob = np.zeros((128, NWBLOB), np.float32)
    wblob[:, OFF_WFT:OFF_WFT + 512] = tiled_T(Wf)
    wblob[:, OFF_WAT:OFF_WAT + 512] = tiled_T(Wa)
    wblob[:, OFF_WBT:OFF_WBT + 512] = tiled_T(Wb * 0.5)
    wcr = np.broadcast_to((Wc.ravel() * 0.5).reshape(2, 128, 1),
                          (2, 128, 128)).transpose(1, 0, 2)
    wblob[:, OFF_WCR:OFF_WCR + 256] = wcr.reshape(128, 256)
    wblob[:, OFF_BFR:OFF_BFR + 256] = bf[None, :]
    wblob[:, OFF_BAR:OFF_BAR + 256] = ba[None, :]
    wblob[:, OFF_BBR:OFF_BBR + 256] = (bb * 0.5)[None, :]
    wblob[:, OFF_ONE:OFF_ONE + 2] = 1.0
    sblob_base = np.zeros((128, NSBLOB), np.float32)
    sblob_base[:, OFF_B1C:OFF_B1C + 2] = tiled_v(b1)

    # W1.T tiled: [p, c*256+m] = W1[m, c*128+p]
    w1tt = np.ascontiguousarray(
        W1.T.reshape(KC, 128, DHID).transpose(1, 0, 2)
    ).reshape(128, KC * DHID).astype(ml_dtypes.bfloat16)

    relu_b1 = np.maximum(b1, 0.0).astype(np.float32)

    in_maps = []
    for k in range(N_CORES):
        rows = np.nonzero(cid == k)[0]
        nk = len(rows)
        shard = np.zeros((npad, DIN), dtype=ml_dtypes.bfloat16)
        shard[:nk] = xb16[rows]
        # per block: [sz, 1024] -> [p, c, r]; concat along cols
        parts, off = [], 0
        for sz in sizes:
            parts.append(shard[off:off + sz].reshape(sz, KC, 128)
                         .transpose(2, 1, 0).reshape(128, KC * sz))
            off += sz
        xcore = np.ascontiguousarray(np.concatenate(parts, axis=1))

        invc = np.float32(1.0 / max(float(counts[k]), 1.0))
        n_pad = float(npad - nk)
        sblob_k = sblob_base.copy()
        corr = tiled_v((-invc * n_pad) * relu_b1)     # [128, 2]
        sblob_k[:, OFF_CORR:OFF_CORR + 4] = corr[:, [0, 0, 1, 1]]
        sblob_k[:, OFF_INVC] = invc
        in_maps.append({"xb": xcore, "w1t": w1tt, "wblob": wblob,
                        "sblob": sblob_k})
    return in_maps, sizes


def kernel(**inputs):
    _, _, sizes = _shard_plan(inputs["cluster_id"])
    key = ("nc", sizes)
    if key not in _CACHE:
        _CACHE[key] = _build_nc(sizes)
        _CACHE["nc"] = _CACHE[key]       # convenience handle for test.py
    nc = _CACHE[key]
    in_maps, _ = _prep_inputs(**inputs)
    res = bass_utils.run_bass_kernel_spmd(
        nc, in_maps, core_ids=list(range(N_CORES)))
    return _combine([res.results[k]["out"] for k in range(N_CORES)])


def _combine(outs):
    """Host-side gather: softmax over per-cluster logits + weighted sum."""
    logits = np.array([float(np.asarray(o)[0, 4]) for o in outs],
                      dtype=np.float64)
    h_path = np.stack([np.asarray(o)[:, [0, 2]].T.reshape(DHID)
                       for o in outs])
    w = np.exp(logits - logits.max())
    w /= w.sum()
    H = (w[:, None] * h_path.astype(np.float64)).sum(axis=0)
    return np.ascontiguousarray(H.reshape(1, DHID)).astype(np.float32)


# revision 29
# speedup vs baseline: 1.2029x; 1.0015x over previous
"""
DeepAttMISL segment-reduce kernel for Trainium2 (Bass/Tile), 8 NeuronCores.

Math (see reference):
  h        = relu(x @ W1.T + b1)                    x:[N,1024] -> h:[N,256]
  seg      = segment_sum(h, cluster_id, 8)          -> [8,256]
  h_clust  = seg / max(counts,1)
  h_path   = relu(h_clust @ Wf.T + bf)
  A        = softmax((tanh(h_path@Wa.T+ba) * sigmoid(h_path@Wb.T+bb)) @ Wc.T)
  H        = A @ h_path                             -> [1,256]

Sharding: BY CLUSTER, not by rows.  Core k receives ALL rows of cluster k
(host sorts rows by cluster_id), zero-padded to a fixed NPAD rows.  Each
core therefore owns its cluster's full segment sum locally and NO cross-core
collective is needed (the ncfw AllReduce costs 25-35us per op in this
runtime, plus a ~56us wake, and dominated the previous version's critical
path).  Each core runs the tiny gated-attention head for its own cluster and
outputs (logit_k, h_path_k); the host does the final 8-way softmax +
weighted sum as the gather/unshard step.

Main matmul is computed TRANSPOSED (W1 stationary, x moving, h.T in PSUM
[hid_half, rows]) so the segment sum falls out of ACT's accum_out: one
activation op per PSUM tile does bias + relu + sum-over-rows.  No segment
matmuls, no one-hot matrix.  Zero-pad rows contribute exactly relu(b1)
each; the host bakes -n_pad*relu(b1)/count into a per-core correction.

x is streamed as per-block contiguous DMAs on the sync ring at line rate
(~425GB/s), with small lead blocks to beat the DMA ramp (per-transfer cold
descriptor + laggard-engine completion skew) and a small tail block to trim
padding.  All DMAs ride the sync ring in consumption order (W1 split so its
first piece eats the cold descriptor; head weights last).  bf16 everywhere
in the big matmul (fp8 fails the 2e-2 gate: W1's quantization error is
shared across instances so it does not average out); float32r (fp22) head
with duplicated-pair columns (the fp32r ISA needs an even moving free dim)
and biases folded in as K=1 matmuls.  PE warm-up bridges the ~13us DMA
lead-in and filler matmuls keep HAM from re-throttling across the head.
sigmoid(y) = 0.5*(1+tanh(y/2)) with the 0.5 folded into Wc so one ACT
table set (relu/tanh/exp) serves the whole kernel.
"""

import sys

if "/opt/trn_rl_repo" not in sys.path:
    sys.path.insert(0, "/opt/trn_rl_repo")

import numpy as np
import ml_dtypes

import concourse.bass as bass
import concourse.tile as tile
from concourse import bacc, mybir
from concourse import bass_utils

ALU = mybir.AluOpType

N_CORES = 8
N_TOTAL = 65536
DIN = 1024
DHID = 256
K_CL = 8
KC = DIN // 128                        # 8 contraction chunks of 128
BLK = 448                              # bulk rows per block (<=512 fp32 PSUM
                                       # bank; 448 cuts zero-padding to ~2%)
BLK0 = 224                             # first/last block halved: block 0
                                       # completes its DMA sooner (earlier PE
                                       # start), same total padding
WARMUP_MMS = 15                        # PE bridge: engine free ~7.4us, block0
                                       # ~13us; also >=3.4us busy for HAM

# wblob: head weights, float32r (PE-only consumers), per-partition fp32 elems
OFF_WFT = 0                            # Wf.T tiled   [2,256] -> 512
OFF_WAT = 512                          # Wa.T tiled   [2,256] -> 512
OFF_WBT = 1024                         # (Wb/2).T     [2,256] -> 512
OFF_WCR = 1536                         # (Wc/2) bcast [2,128] -> 256
OFF_BFR = 1792                         # bf as a row (bcast to all p) -> 256
OFF_BAR = 2048                         # ba row                       -> 256
OFF_BBR = 2304                         # bb/2 row                     -> 256
OFF_ONE = 2560                         # constant 1.0 x2 (rhs of bias MMs)
NWBLOB = 2562
# sblob: fp32 scalars for ACT bias / DVE (b1 tiled, padding fix, 1/count)
OFF_B1C = 0                            # b1 tiled [2]
OFF_CORR = 2                           # -invc*n_pad*relu(b1), dup pairs [4]
OFF_INVC = 6                           # 1/max(count,1) scalar [1]
NSBLOB = 7

BF16 = mybir.dt.bfloat16
F32 = mybir.dt.float32
AF = mybir.ActivationFunctionType

_CACHE = {}


def _build_nc(sizes):
    nblk = len(sizes)
    offs = [0]
    for s in sizes:
        offs.append(offs[-1] + s)
    tot = offs[-1]
    nc = bacc.Bacc("TRN2", target_bir_lowering=False, debug=False,
                   num_devices=N_CORES)

    xb = nc.dram_tensor("xb", [128, KC * tot], BF16,
                        kind="ExternalInput")
    w1t = nc.dram_tensor("w1t", [128, KC * DHID], BF16, kind="ExternalInput")
    wblob = nc.dram_tensor("wblob", [128, NWBLOB], mybir.dt.float32r,
                           kind="ExternalInput")
    sblob = nc.dram_tensor("sblob", [128, NSBLOB], F32, kind="ExternalInput")
    out = nc.dram_tensor("out", [128, 6], F32, kind="ExternalOutput")

    with tile.TileContext(nc) as tc:
        with tc.tile_pool(name="consts", bufs=1) as consts, \
             tc.tile_pool(name="xblk", bufs=1) as xblk, \
             tc.tile_pool(name="hps", bufs=4, space="PSUM") as hps, \
             tc.tile_pool(name="headps", bufs=2, space="PSUM") as headps, \
             tc.tile_pool(name="small", bufs=1) as small:

            # ---- PE warm-up bridge: keep HAM busy (and un-throttled by the
            # time real data arrives) from t~0 until block 0 lands (~13us).
            wz = consts.tile([128, BLK], BF16)
            nc.vector.memset(wz[:], 0.0)
            wps = hps.tile([128, BLK], F32, tag="main")
            for _ in range(WARMUP_MMS):
                nc.tensor.matmul(wps[:], wz[:, 0:128], wz[:],
                                 start=True, stop=True, skip_group_check=True)

            # ---- DMAs: ALL on the sync ring, in consumption order.  The
            # tiny sblob goes first purely to eat the ~1us cold-first-
            # descriptor cost on all 16 SDMA engines; then W1 (gates the
            # first MM), then the x blocks as back-to-back ~0.9MiB
            # transfers (7KiB/partition lines, full line rate), and the
            # head-weight blob LAST (needed only ~60us in; anywhere earlier
            # it steals SDMA bandwidth from block 0 and delays the whole
            # pipeline).
            F32R = mybir.dt.float32r
            w1t_sb = consts.tile([128, KC, DHID], BF16)
            w1v = w1t.ap().rearrange("p (k f) -> p k f", k=KC)
            # W1 in two pieces: the first eats the SDMA engines' cold
            # first-descriptor latency (~0.6-1us) while doing useful work
            nc.sync.dma_start(w1t_sb[:, 0:2], w1v[:, 0:2])
            nc.sync.dma_start(w1t_sb[:, 2:KC], w1v[:, 2:KC])

            xts = xblk.tile([128, KC * tot], BF16)
            sblob_sb = consts.tile([128, NSBLOB], F32)
            for b in range(nblk):
                nc.sync.dma_start(
                    xts[:, KC * offs[b]:KC * offs[b + 1]],
                    xb.ap()[:, KC * offs[b]:KC * offs[b + 1]])
                if b == 0:
                    nc.sync.dma_start(sblob_sb[:], sblob.ap())
            wblob_sb = consts.tile([128, NWBLOB], F32R)
            nc.sync.dma_start(wblob_sb[:], wblob.ap())

            # ---- main loop: per block, per hid-half: 8 accumulated MMs
            # (W1 chunk stationary, x moving, N=512), then one ACT op doing
            # bias + relu + accum_out (the segment sum over this block).
            segparts = small.tile([128, 2, nblk], F32)
            seghalf = small.tile([128, 2], F32)
            hsc = [small.tile([128, BLK], BF16, name=f"hsc{i}")
                   for i in range(3)]
            bhalf = nblk // 2
            for b in range(nblk):
                sz, off = sizes[b], offs[b]
                for j in range(2):
                    ps = hps.tile([128, sz], F32, tag="main",
                                  padded_shape=[128, 512])
                    for c in range(KC):
                        nc.tensor.matmul(
                            ps[:],
                            w1t_sb[:, c, j * 128:(j + 1) * 128],
                            xts[:, KC * off + c * sz:KC * off + (c + 1) * sz],
                            start=(c == 0), stop=(c == KC - 1),
                            skip_group_check=True)
                    nc.scalar.activation(
                        hsc[(2 * b + j) % 3][:, 0:sz], ps[:], AF.Relu,
                        bias=sblob_sb[:, OFF_B1C + j:OFF_B1C + j + 1],
                        accum_out=segparts[:, j, b:b + 1])
                if b == bhalf:
                    # fold the first half of the block partials early, off
                    # the critical tail (DVE is idle mid-loop)
                    for j in range(2):
                        nc.vector.reduce_sum(seghalf[:, j:j + 1],
                                             segparts[:, j, 0:bhalf + 1],
                                             axis=mybir.AxisListType.X)

            # ---- local segment sum -> cluster mean (with padding fix).
            # Columns come in duplicated pairs (cols 2j and 2j+1 equal):
            # the fp32r matmul ISA requires an even moving free dim, so the
            # whole head works on [128, 4] with N=2 GEMVs.
            seg4 = small.tile([128, 4], F32)
            for j in range(2):
                nc.vector.reduce_sum(seg4[:, 2 * j:2 * j + 1],
                                     segparts[:, j, bhalf + 1:],
                                     axis=mybir.AxisListType.X)
            for j in range(2):
                nc.vector.tensor_add(seg4[:, 2 * j:2 * j + 1],
                                     seg4[:, 2 * j:2 * j + 1],
                                     seghalf[:, j:j + 1])
                nc.vector.tensor_copy(seg4[:, 2 * j + 1:2 * j + 2],
                                      seg4[:, 2 * j:2 * j + 1])
            hc = small.tile([128, 4], F32R)
            nc.vector.tensor_scalar_mul(hc[:], seg4[:],
                                        sblob_sb[:, OFF_INVC:OFF_INVC + 1])
            nc.vector.tensor_add(hc[:], hc[:],
                                 sblob_sb[:, OFF_CORR:OFF_CORR + 4])

            # ---- gated-attention head for this core's cluster ----
            # fp32r matmuls (1-pass fp22, single LDW pass vs fp32's two of
            # each); bias folded into the accumulation group as a K=1
            # matmul against a constant-one rhs so one ACT handles both
            # hid-halves of a layer.
            def head_layer(w_off, b_off, rhs, func, name):
                o = small.tile([128, 4], F32R, name=name)
                ps = headps.tile([128, 4], F32, tag="head",
                                 padded_shape=[128, BLK])
                n = 0
                for j in range(2):
                    for i in range(2):
                        nc.tensor.matmul(
                            ps[:, 2 * j:2 * j + 2],
                            wblob_sb[:, w_off + i * 256 + j * 128:
                                     w_off + i * 256 + (j + 1) * 128],
                            rhs[:, 2 * i:2 * i + 2],
                            start=(n == 0), stop=False,
                            skip_group_check=True)
                        n += 1
                    nc.tensor.matmul(
                        ps[:, 2 * j:2 * j + 2],
                        wblob_sb[0:1, b_off + j * 128:
                                 b_off + (j + 1) * 128],
                        wblob_sb[0:1, OFF_ONE:OFF_ONE + 2],
                        start=False, stop=(j == 1), skip_group_check=True)
                nc.scalar.activation(o[:], ps[:], func)
                return o

            fps = headps.tile([128, 2], F32, tag="fill",
                              padded_shape=[128, 512])

            def pe_filler(n):
                # dummy f32r MMs with no data deps: they run while the next
                # layer waits on its ACT, keeping HAM from re-throttling
                for _ in range(n):
                    nc.tensor.matmul(
                        fps[:], wblob_sb[:, 0:128],
                        wblob_sb[:, OFF_ONE:OFF_ONE + 2],
                        start=True, stop=True, skip_group_check=True)

            pe_filler(12)
            hpT = head_layer(OFF_WFT, OFF_BFR, hc, AF.Relu, "hpT")
            pe_filler(6)
            aT = head_layer(OFF_WAT, OFF_BAR, hpT, AF.Tanh, "aT")
            tT = head_layer(OFF_WBT, OFF_BBR, hpT, AF.Tanh, "tT")
            pe_filler(6)
            # a*g = 0.5*a*(1+tanh(y/2)); the 0.5 lives in Wc/2
            ag = small.tile([128, 4], F32R)
            nc.vector.tensor_mul(ag[:], aT[:], tT[:])
            nc.vector.tensor_add(ag[:], ag[:], aT[:])

            # logit (replicated across partitions via broadcast Wc/2)
            lps = headps.tile([128, 2], F32, tag="head",
                              padded_shape=[128, BLK])
            for j in range(2):
                nc.tensor.matmul(
                    lps[:],
                    wblob_sb[:, OFF_WCR + j * 128:
                             OFF_WCR + (j + 1) * 128],
                    ag[:, 2 * j:2 * j + 2],
                    start=(j == 0), stop=(j == 1))

            # stream h_path out as soon as it's ready (overlaps the gate
            # matmuls); the logit follows in a second small DMA
            nc.sync.dma_start(out.ap()[:, 0:4].bitcast(F32R), hpT[:])
            lsb = small.tile([128, 2], F32)
            nc.vector.tensor_copy(lsb[:], lps[:])
            nc.sync.dma_start(out.ap()[:, 4:6], lsb[:])

    nc.compile()
    return nc


def _shard_plan(cluster_id):
    cid = np.asarray(cluster_id).astype(np.int64).reshape(N_TOTAL)
    counts = np.bincount(cid, minlength=K_CL).astype(np.int64)
    mx = int(counts.max())
    # small lead blocks: the DMA stream ramps slowly (per-transfer cold
    # descriptor + one laggard SDMA engine gates each completion), so early
    # blocks complete sooner and the PE pipeline starts ~2us earlier; a
    # small tail block also trims zero-padding to <=447 rows of quantum 112
    lead = (112, 112) + (224,) * 6
    k = max(0, -(-(mx - sum(lead) - 112) // BLK))
    sizes = lead + (BLK,) * k + (112,)
    return cid, counts, sizes


def _prep_inputs(x_path, cluster_id, W1, b1, Wf, bf, Wa, ba, Wb, bb, Wc, bc):
    """Host-side sharding / marshalling. Returns (in_maps, sizes)."""
    cid, counts, sizes = _shard_plan(cluster_id)
    npad = sum(sizes)
    x = np.asarray(x_path, dtype=np.float32).reshape(N_TOTAL, DIN)
    xb16 = x.astype(ml_dtypes.bfloat16)

    W1 = np.asarray(W1, np.float32); b1 = np.asarray(b1, np.float32)
    Wf = np.asarray(Wf, np.float32); bf = np.asarray(bf, np.float32)
    Wa = np.asarray(Wa, np.float32); ba = np.asarray(ba, np.float32)
    Wb = np.asarray(Wb, np.float32); bb = np.asarray(bb, np.float32)
    Wc = np.asarray(Wc, np.float32)

    def tiled_T(M):  # [256,256] -> [128, 512]; [p, j*256+f] = M.T[j*128+p, f]
        return np.ascontiguousarray(
            M.T.reshape(2, 128, DHID).transpose(1, 0, 2)).reshape(128, 512)

    def tiled_v(v):  # [256] -> [128, 2]; [p, j] = v[j*128+p]
        return np.ascontiguousarray(v.reshape(2, 128).T)

    wblob = np.zeros((128, NWBLOB), np.float32)
    wblob[:, OFF_WFT:OFF_WFT + 512] = tiled_T(Wf)
    wblob[:, OFF_WAT:OFF_WAT + 512] = tiled_T(Wa)
    wblob[:, OFF_WBT:OFF_WBT + 512] = tiled_T(Wb * 0.5)
    wcr = np.broadcast_to((Wc.ravel() * 0.5).reshape(2, 128, 1),
                          (2, 128, 128)).transpose(1, 0, 2)
    wblob[:, OFF_WCR:OFF_WCR + 256] = wcr.reshape(128, 256)
    wblob[:, OFF_BFR:OFF_BFR + 256] = bf[None, :]
    wblob[:, OFF_BAR:OFF_BAR + 256] = ba[None, :]
    wblob[:, OFF_BBR:OFF_BBR + 256] = (bb * 0.5)[None, :]
    wblob[:, OFF_ONE:OFF_ONE + 2] = 1.0
    sblob_base = np.zeros((128, NSBLOB), np.float32)
    sblob_base[:, OFF_B1C:OFF_B1C + 2] = tiled_v(b1)

    # W1.T tiled: [p, c*256+m] = W1[m, c*128+p]
    w1tt = np.ascontiguousarray(
        W1.T.reshape(KC, 128, DHID).transpose(1, 0, 2)
    ).reshape(128, KC * DHID).astype(ml_dtypes.bfloat16)

    relu_b1 = np.maximum(b1, 0.0).astype(np.float32)

    in_maps = []
    for k in range(N_CORES):
        rows = np.nonzero(cid == k)[0]
        nk = len(rows)
        shard = np.zeros((npad, DIN), dtype=ml_dtypes.bfloat16)
        shard[:nk] = xb16[rows]
        # per block: [sz, 1024] -> [p, c, r]; concat along cols
        parts, off = [], 0
        for sz in sizes:
            parts.append(shard[off:off + sz].reshape(sz, KC, 128)
                         .transpose(2, 1, 0).reshape(128, KC * sz))
            off += sz
        xcore = np.ascontiguousarray(np.concatenate(parts, axis=1))

        invc = np.float32(1.0 / max(float(counts[k]), 1.0))
        n_pad = float(npad - nk)
        sblob_k = sblob_base.copy()
        corr = tiled_v((-invc * n_pad) * relu_b1)     # [128, 2]
        sblob_k[:, OFF_CORR:OFF_CORR + 4] = corr[:, [0, 0, 1, 1]]
        sblob_k[:, OFF_INVC] = invc
        in_maps.append({"xb": xcore, "w1t": w1tt, "wblob": wblob,
                        "sblob": sblob_k})
    return in_maps, sizes


def kernel(**inputs):
    _, _, sizes = _shard_plan(inputs["cluster_id"])
    key = ("nc", sizes)
    if key not in _CACHE:
        _CACHE[key] = _build_nc(sizes)
        _CACHE["nc"] = _CACHE[key]       # convenience handle for test.py
    nc = _CACHE[key]
    in_maps, _ = _prep_inputs(**inputs)
    res = bass_utils.run_bass_kernel_spmd(
        nc, in_maps, core_ids=list(range(N_CORES)))
    return _combine([res.results[k]["out"] for k in range(N_CORES)])


def _combine(outs):
    """Host-side gather: softmax over per-cluster logits + weighted sum."""
    logits = np.array([float(np.asarray(o)[0, 4]) for o in outs],
                      dtype=np.float64)
    h_path = np.stack([np.asarray(o)[:, [0, 2]].T.reshape(DHID)
                       for o in outs])
    w = np.exp(logits - logits.max())
    w /= w.sum()
    H = (w[:, None] * h_path.astype(np.float64)).sum(axis=0)
    return np.ascontiguousarray(H.reshape(1, DHID)).astype(np.float32)
